# revision 9
# baseline (speedup 1.0000x reference)
"""Trainium2 Bass kernel for nn_LowFreqCrossAttn (dense transformer cross-attention).

Data-parallel over batch: 16 batches -> 8 NeuronCores, 2 batches/core.
Weights / attention-bias tables replicated.

Per-core dataflow (all matmuls fp16 x fp16 -> f32 PSUM):
  A) q = (s*Wq) @ ll, k = (0.5*Wk) @ ha   (head rows duplicated to K=128:
     K<128 matmul streams never warm the PE HAM clock gate -> half clock)
     vT = ha^T @ WvT (+bias row)          (token-major, dense 48-col head
     blocks, re-laid to 80-col blocks with a ones col @64)
  B) per (head, batch): logitsT = k_h^T q_h  (m on partitions, n free)
     e = exp(logitsT) * exp_bias^T  (ACT exp -> f16, DVE/GPS mult; no
     max-subtraction: |logits + bias| <= 1.1 for this model)
     out_unT[d, n] (+ s row @ partition 64) = vT_slice^T @ e  (PSUM accum
     over m-tiles; QK + AV psum tiles span 2 banks so exp/evac run as one
     fat strided op per (h, b, m) instead of per chunk)
  B-tail, per batch) s rows -> DRAM bounce -> [16, 392] -> one batched DVE
     reciprocal -> DRAM bounce -> [1, 6272] -> gpsimd partition_broadcast
     out_norm = out_unT * (1/s)  (pair tiles, c-major, f16)
  C) y = WpT^T @ out_norm + b  (channel-major f32 out)
"""

import numpy as np

B = 16
C = 384
RES = 28
N = 784
NH = 8
HD = 48
NP = 392            # n-chunk (half of N; fits one PSUM bank in f32)
NCORES = 8
BPC = 2             # batches per core
SCALE = HD ** -0.5
# m-tiles: 6 x 128 + 1 x 16 (K=128 keeps the PE HAM warm)
MTS = [(128 * i, 128) for i in range(6)] + [(768, 16)]
NMT = len(MTS)

TRACE = False       # set True to capture an NTFF trace on core 0
LAST_RESULTS = {}   # exec_time_ns etc. from the last run (when TRACE)

_CACHE = {}


def _build_nc():
    import concourse.bacc as bacc
    import concourse.mybir as mybir
    import concourse.tile as tile

    f16 = mybir.dt.float16
    f32 = mybir.dt.float32
    AF = mybir.ActivationFunctionType
    MUL = mybir.AluOpType.mult

    nc = bacc.Bacc("TRN2", target_bir_lowering=False, debug=False)

    ll_d = nc.declare_dram_parameter("ll", [BPC, C, N], f16, isOutput=False)
    ha_d = nc.declare_dram_parameter("ha", [BPC, C, N], f16, isOutput=False)
    qwT_d = nc.declare_dram_parameter("qwT", [3, 128, 1024], f16, isOutput=False)
    kwT_d = nc.declare_dram_parameter("kwT", [3, 128, 1024], f16, isOutput=False)
    vwT_d = nc.declare_dram_parameter("vwT", [3, 128, 384], f16, isOutput=False)
    pwT_d = nc.declare_dram_parameter("pwT", [4, 128, 384], f16, isOutput=False)
    qb_d = nc.declare_dram_parameter("qb", [128, 8], f32, isOutput=False)
    kb_d = nc.declare_dram_parameter("kb", [128, 8], f32, isOutput=False)
    vb_d = nc.declare_dram_parameter("vb", [1, 384], f16, isOutput=False)
    pb_d = nc.declare_dram_parameter("pb", [128, 3], f32, isOutput=False)
    expb_d = nc.declare_dram_parameter("expb", [NH, N, N], f16, isOutput=False)
    vinit_d = nc.declare_dram_parameter("vinit", [128, 640], f16, isOutput=False)
    out_d = nc.declare_dram_parameter("out", [BPC, C, N], f32, isOutput=True)

    with tile.TileContext(nc) as tc:
        with (
            tc.tile_pool(name="const", bufs=1) as cp,
            tc.tile_pool(name="persist", bufs=1) as pp,
            tc.tile_pool(name="dram", bufs=1, space="DRAM") as dp,
        ):
            # ---- load constants ----
            qwT_sb = [cp.tile([128, 1024], f16, tag=f"qwT{t}", name=f"qwT{t}") for t in range(3)]
            kwT_sb = [cp.tile([128, 1024], f16, tag=f"kwT{t}", name=f"kwT{t}") for t in range(3)]
            vwT_sb = [cp.tile([128, 384], f16, tag=f"vwT{t}", name=f"vwT{t}") for t in range(3)]
            pwT_sb = [cp.tile([128, 384], f16, tag=f"pwT{p}", name=f"pwT{p}") for p in range(4)]
            for t in range(3):
                nc.sync.dma_start(qwT_sb[t][:], qwT_d[t])
                nc.sync.dma_start(kwT_sb[t][:], kwT_d[t])
                nc.sync.dma_start(vwT_sb[t][:], vwT_d[t])
            for p in range(4):
                nc.sync.dma_start(pwT_sb[p][:], pwT_d[p])
            qb_sb = cp.tile([128, 8], f32, tag="qb", name="qb")
            kb_sb = cp.tile([128, 8], f32, tag="kb", name="kb")
            vb_sb = cp.tile([1, 384], f16, tag="vb", name="vb")
            pb_sb = cp.tile([128, 3], f32, tag="pb", name="pb")
            nc.sync.dma_start(qb_sb[:], qb_d[:])
            nc.sync.dma_start(kb_sb[:], kb_d[:])
            nc.sync.dma_start(vb_sb[:], vb_d[:])
            nc.sync.dma_start(pb_sb[:], pb_d[:])
            ones128 = cp.tile([1, 128], f16, tag="ones128", name="ones128")
            nc.gpsimd.memset(ones128[:], 1.0)

            # ---- persistent activation tiles ----
            q_sb = [[pp.tile([128, N], f16, tag=f"q{b}_{h}", name=f"q{b}_{h}")
                     for h in range(NH)] for b in range(BPC)]
            k_sb = [[pp.tile([128, N], f16, tag=f"k{b}_{h}", name=f"k{b}_{h}")
                     for h in range(NH)] for b in range(BPC)]
            vT_sb = [[pp.tile([128, 640], f16, tag=f"vT{b}_{m}", name=f"vT{b}_{m}")
                      for m in range(NMT)] for b in range(BPC)]
            ounT = [pp.tile([65, NH, N], f16, tag=f"ounT{b}", name=f"ounT{b}")
                    for b in range(BPC)]
            onorm = [[pp.tile([128, N], f16, tag=f"onorm{b}_{p}", name=f"onorm{b}_{p}")
                      for p in range(4)] for b in range(BPC)]
            s_all = [pp.tile([16, NP], f16, tag=f"s{b}", name=f"s{b}") for b in range(BPC)]
            r_all = [pp.tile([16, NP], f32, tag=f"r{b}", name=f"r{b}") for b in range(BPC)]
            r16 = [pp.tile([16, NP], f16, tag=f"r16{b}", name=f"r16{b}")
                   for b in range(BPC)]
            bc_all = [pp.tile([48, NH * N], f16, tag=f"bc{b}", name=f"bc{b}")
                      for b in range(BPC)]
            sg_dram = [dp.tile([16, NP], f16, tag=f"sg{b}", name=f"sg{b}")
                       for b in range(BPC)]
            r_dram = [dp.tile([16, NP], f16, tag=f"rd{b}", name=f"rd{b}")
                      for b in range(BPC)]

            # one-time layout init: vT 80-blocks (zeros + ones col @64) via DMA
            # const; onorm pad rows zeroed (32-aligned; data rows rewritten later)
            for b in range(BPC):
                for m in range(NMT):
                    nc.sync.dma_start(vT_sb[b][m][:], vinit_d[:])
                for p in range(4):
                    nc.gpsimd.memset(onorm[b][p][32:64, :], 0.0)
                    nc.gpsimd.memset(onorm[b][p][96:128, :], 0.0)

            # ---- phase A: projections ----
            with (
                tc.tile_pool(name="actA", bufs=1) as apool,
                tc.tile_pool(name="psA", bufs=2, space="PSUM") as psA,
            ):
                ll_sb = [[apool.tile([128, N], f16, tag=f"ll{b}_{t}", name=f"ll{b}_{t}")
                          for t in range(3)] for b in range(BPC)]
                ha_sb = [[apool.tile([128, N], f16, tag=f"ha{b}_{t}", name=f"ha{b}_{t}")
                          for t in range(3)] for b in range(BPC)]
                for b in range(BPC):
                    for t in range(3):
                        nc.sync.dma_start(ll_sb[b][t][:], ll_d[b, 128 * t:128 * (t + 1), :])
                        nc.sync.dma_start(ha_sb[b][t][:], ha_d[b, 128 * t:128 * (t + 1), :])
                    # q / k projections -> duplicated per-head tiles (rows 0-63 == 64-127)
                    for (wt, bt, src_, dst) in (
                        (qwT_sb, qb_sb, ll_sb[b], q_sb[b]),
                        (kwT_sb, kb_sb, ha_sb[b], k_sb[b]),
                    ):
                        for h in range(NH):
                            ps = psA.tile([128, 1024], f32, tag="qk", name="psqk")
                            for nch in range(2):
                                for t in range(3):
                                    nc.tensor.matmul(
                                        ps[:, 512 * nch:512 * nch + NP],
                                        wt[t][:, 128 * h:128 * (h + 1)],
                                        src_[t][:, NP * nch:NP * (nch + 1)],
                                        start=(t == 0),
                                        stop=(t == 2),
                                    )
                            nc.scalar.activation(
                                dst[h].rearrange("p (c n) -> p c n", c=2),
                                ps.rearrange("p (c n) -> p c n", n=512)[:, :, 0:NP],
                                AF.Identity, bias=bt[:, h:h + 1],
                            )
                    # vT projection -> dense 48-blocks, strided copy to 80-blocks
                    for mi, (off, msz) in enumerate(MTS):
                        ps = psA.tile([128, 384], f32, tag="vt", name="psvt")
                        nc.tensor.matmul(ps[0:msz, :], ones128[:, 0:msz], vb_sb[:],
                                         start=True, stop=False)
                        for t in range(3):
                            nc.tensor.matmul(
                                ps[0:msz, :],
                                ha_sb[b][t][:, off:off + msz],
                                vwT_sb[t][:],
                                start=False,
                                stop=(t == 2),
                            )
                        nc.vector.tensor_copy(
                            vT_sb[b][mi].rearrange("p (h c) -> p h c", c=80)[0:msz, :, 0:48],
                            ps.rearrange("p (h c) -> p h c", c=48)[0:msz],
                        )

            # ---- phase B: attention ----
            with (
                tc.tile_pool(name="ebp", bufs=3) as ebp,
                tc.tile_pool(name="etp", bufs=2) as etp,
                tc.tile_pool(name="psqk", bufs=2, space="PSUM") as psqk,
                tc.tile_pool(name="psav", bufs=2, space="PSUM") as psav,
            ):
                eb_tiles = {}

                def attend(h, b):
                    eb_sb = eb_tiles[h]
                    av = psav.tile([65, 1024], f32, tag="av", name="avt")
                    for mi, (off, msz) in enumerate(MTS):
                        eT = etp.tile([128, N], f16, tag="eT", bufs=4, name="eTt")
                        qk = psqk.tile([128, 1024], f32, tag="qk", name="qkt")
                        for nch in range(2):
                            nc.tensor.matmul(
                                qk[0:msz, 512 * nch:512 * nch + NP],
                                k_sb[b][h][:, off:off + msz],
                                q_sb[b][h][:, NP * nch:NP * (nch + 1)],
                                start=True, stop=True,
                            )
                        nc.scalar.activation(
                            eT[0:msz].rearrange("p (c n) -> p c n", c=2),
                            qk[0:msz].rearrange("p (c n) -> p c n", n=512)[:, :, 0:NP],
                            AF.Exp)
                        # exp_bias multiply: split DVE / GPSIMD by m-tile
                        eng = nc.gpsimd if mi in (1, 4) else nc.vector
                        eng.tensor_tensor(
                            eT[0:msz, :], eT[0:msz, :], eb_sb[0:msz, mi, :], MUL)
                        for nch in range(2):
                            nc.tensor.matmul(
                                av[:, 512 * nch:512 * nch + NP],
                                vT_sb[b][mi][0:msz, 80 * h:80 * h + 65],
                                eT[0:msz, NP * nch:NP * (nch + 1)],
                                start=(mi == 0), stop=(mi == NMT - 1),
                            )
                    # evacuate out_unT + s row (f16)
                    nc.vector.tensor_copy(
                        ounT[b][:, h, :].rearrange("p (c n) -> p c n", c=2),
                        av.rearrange("p (c n) -> p c n", n=512)[:, :, 0:NP],
                    )

                def tail(b):
                    # batched softmax denominators + normalization
                    nc.sync.dma_start(
                        sg_dram[b].rearrange("p n -> () (p n)").rearrange(
                            "() (h n) -> () h n", n=N),
                        ounT[b][64:65, :, :],
                    )
                    nc.sync.dma_start(s_all[b][:], sg_dram[b][:])
                    nc.vector.reciprocal(r_all[b][:], s_all[b][:])
                    nc.vector.tensor_copy(r16[b][:], r_all[b][:])
                    nc.sync.dma_start(r_dram[b][:], r16[b][:])
                    nc.sync.dma_start(
                        bc_all[b][:],
                        r_dram[b].tensor.ap().rearrange(
                            "p n -> () (p n)").to_broadcast((48, NH * N)),
                    )
                    for h in range(NH):
                        prr, hpp = divmod(h, 2)
                        nc.vector.tensor_tensor(
                            onorm[b][prr][64 * hpp:64 * hpp + 48, :],
                            ounT[b][0:48, h, :],
                            bc_all[b][:, N * h:N * (h + 1)],
                            MUL,
                        )

                def proj(b):
                    for o in range(3):
                        ps = psqk.tile([128, 1024], f32, tag="qk", name="psy")
                        for nch in range(2):
                            for p in range(4):
                                nc.tensor.matmul(
                                    ps[:, 512 * nch:512 * nch + NP],
                                    pwT_sb[p][:, 128 * o:128 * (o + 1)],
                                    onorm[b][p][:, NP * nch:NP * (nch + 1)],
                                    start=(p == 0), stop=(p == 3),
                                )
                        y_sb = ypool.tile([128, N], f32, tag="y", name="ysb")
                        nc.scalar.activation(
                            y_sb.rearrange("p (c n) -> p c n", c=2),
                            ps.rearrange("p (c n) -> p c n", n=512)[:, :, 0:NP],
                            AF.Identity, bias=pb_sb[:, o:o + 1])
                        nc.sync.dma_start(
                            out_d[b, 128 * o:128 * (o + 1), :], y_sb[:])

                with tc.tile_pool(name="yp", bufs=3) as ypool:
                    # b=1 lags one head so b=0's tail overlaps b=1's last heads
                    sched = []
                    for h in range(NH):
                        sched.append((h, 0))
                        if h >= 1:
                            sched.append((h - 1, 1))
                    sched.append((NH - 1, 1))
                    for (h, b) in sched:
                        if b == 0 and h not in eb_tiles:
                            eb_sb = ebp.tile([128, NMT, N], f16, tag="eb", name="ebt")
                            for mi, (off, msz) in enumerate(MTS):
                                nc.sync.dma_start(eb_sb[0:msz, mi, :],
                                                  expb_d[h, off:off + msz, :])
                            eb_tiles[h] = eb_sb
                        attend(h, b)
                        if h == NH - 1:
                            tail(b)
                            proj(b)

    nc.finalize()
    return nc


def _prep_consts(q_w, q_b, kv_w, kv_b, proj_w, proj_b, attn_biases, bias_idxs):
    f16 = np.float16
    qw = (q_w * SCALE).astype(np.float32)
    qb = (q_b * SCALE).astype(np.float32)
    kw = kv_w[:C] * 0.5
    kb = kv_b[:C] * 0.5
    vw = kv_w[C:]
    vb = kv_b[C:]

    def pad64(w2, b1):  # [384(o), 384(c)] -> [512, 384] / [512]
        wp = np.zeros((512, C), np.float32)
        bp = np.zeros((512,), np.float32)
        for h in range(NH):
            wp[64 * h:64 * h + HD] = w2[HD * h:HD * (h + 1)]
            bp[64 * h:64 * h + HD] = b1[HD * h:HD * (h + 1)]
        return wp, bp

    qwp, qbp = pad64(qw, qb)
    kwp, kbp = pad64(kw, kb)

    def dup(w, b1):  # [512, C] -> [1024, C]: per-head 64-block duplicated
        wd = np.zeros((1024, C), np.float32)
        bd = np.zeros((1024,), np.float32)
        for h in range(NH):
            for r in range(2):
                wd[128 * h + 64 * r:128 * h + 64 * r + 64] = w[64 * h:64 * h + 64]
                bd[128 * h + 64 * r:128 * h + 64 * r + 64] = b1[64 * h:64 * h + 64]
        return wd, bd

    qwd, qbd = dup(qwp, qbp)
    kwd, kbd = dup(kwp, kbp)
    qwT = np.ascontiguousarray(qwd.T.reshape(3, 128, 1024)).astype(f16)
    kwT = np.ascontiguousarray(kwd.T.reshape(3, 128, 1024)).astype(f16)
    vwT = np.ascontiguousarray(vw.T.reshape(3, 128, C)).astype(f16)

    # proj weights in onorm pair-tile layout: pair p row j -> channel
    pwT = np.zeros((4, 128, C), np.float32)
    for p in range(4):
        pwT[p, 0:HD] = proj_w[:, 96 * p:96 * p + HD].T
        pwT[p, 64:64 + HD] = proj_w[:, 96 * p + HD:96 * p + 96].T
    pwT = pwT.astype(f16)

    qb_h = np.ascontiguousarray(qbd.reshape(8, 128).T).astype(np.float32)
    kb_h = np.ascontiguousarray(kbd.reshape(8, 128).T).astype(np.float32)
    pb_h = np.ascontiguousarray(proj_b.reshape(3, 128).T).astype(np.float32)
    vb_h = vb.reshape(1, C).astype(f16)

    expb = np.ascontiguousarray(np.exp(attn_biases[:, bias_idxs]).astype(f16))

    vinit = np.zeros((128, 640), f16)
    vinit[:, 64::80] = 1.0

    return dict(qwT=qwT, kwT=kwT, vwT=vwT, pwT=pwT, qb=qb_h, kb=kb_h,
                vb=vb_h, pb=pb_h, expb=expb, vinit=vinit)


def kernel(ll, high_attn, q_w, q_b, kv_w, kv_b, proj_w, proj_b,
           attn_biases, bias_idxs):
    from concourse.bass_utils import run_bass_kernel_spmd

    global LAST_RESULTS
    ll = np.asarray(ll)
    high_attn = np.asarray(high_attn)

    if "nc" not in _CACHE:
        _CACHE["nc"] = _build_nc()
    nc = _CACHE["nc"]

    consts = _prep_consts(
        np.asarray(q_w), np.asarray(q_b), np.asarray(kv_w), np.asarray(kv_b),
        np.asarray(proj_w), np.asarray(proj_b), np.asarray(attn_biases),
        np.asarray(bias_idxs),
    )

    ll16 = ll.reshape(B, C, N).astype(np.float16)
    ha16 = high_attn.reshape(B, C, N).astype(np.float16)

    in_maps = []
    for i in range(NCORES):
        m = {"ll": ll16[BPC * i:BPC * (i + 1)], "ha": ha16[BPC * i:BPC * (i + 1)]}
        m.update(consts)
        in_maps.append(m)

    res = run_bass_kernel_spmd(nc, in_maps, core_ids=list(range(NCORES)),
                               trace=TRACE)
    LAST_RESULTS = {"exec_time_ns": res.exec_time_ns,
                    "scope_times": res.per_core_scope_times}

    out = np.empty((B, C, N), np.float32)
    for i in range(NCORES):
        out[BPC * i:BPC * (i + 1)] = res.results[i]["out"]
    return out.reshape(B, C, RES, RES)


# revision 10
# speedup vs baseline: 1.0409x; 1.0409x over previous
"""Trainium2 Bass kernel for nn_LowFreqCrossAttn (dense transformer cross-attention).

Data-parallel over batch: 16 batches -> 8 NeuronCores, 2 batches/core.
Weights / attention-bias tables replicated.

Per-core dataflow (all matmuls fp16 x fp16 -> f32 PSUM):
  A) q = (s*Wq) @ ll, k = (0.5*Wk) @ ha   (head rows duplicated to K=128:
     K<128 matmul streams never warm the PE HAM clock gate -> half clock)
     vT = ha^T @ WvT (+bias row)          (token-major, dense 48-col head
     blocks, re-laid to 80-col blocks with a ones col @64)
  B) per (head, batch): logitsT = k_h^T q_h  (m on partitions, n free)
     e = exp(logitsT) * exp_bias^T  (ACT exp -> f16, DVE/GPS mult; no
     max-subtraction: |logits + bias| <= 1.1 for this model)
     out_unT[d, n] (+ s row @ partition 64) = vT_slice^T @ e  (PSUM accum
     over m-tiles; QK + AV psum tiles span 2 banks so exp/evac run as one
     fat strided op per (h, b, m) instead of per chunk)
  B-tail, per batch) s rows -> DRAM bounce -> [16, 392] -> one batched DVE
     reciprocal -> DRAM bounce -> [1, 6272] -> gpsimd partition_broadcast
     out_norm = out_unT * (1/s)  (pair tiles, c-major, f16)
  C) y = WpT^T @ out_norm + b  (channel-major f32 out)
"""

import numpy as np

B = 16
C = 384
RES = 28
N = 784
NH = 8
HD = 48
NP = 392            # n-chunk (half of N; fits one PSUM bank in f32)
NCORES = 8
BPC = 2             # batches per core
SCALE = HD ** -0.5
# m-tiles: 6 x 128 + 1 x 16 (K=128 keeps the PE HAM warm)
MTS = [(128 * i, 128) for i in range(6)] + [(768, 16)]
NMT = len(MTS)

TRACE = False       # set True to capture an NTFF trace on core 0
LAST_RESULTS = {}   # exec_time_ns etc. from the last run (when TRACE)

_CACHE = {}


def _build_nc():
    import concourse.bacc as bacc
    import concourse.mybir as mybir
    import concourse.tile as tile

    f16 = mybir.dt.float16
    f32 = mybir.dt.float32
    AF = mybir.ActivationFunctionType
    MUL = mybir.AluOpType.mult

    nc = bacc.Bacc("TRN2", target_bir_lowering=False, debug=False)

    ll_d = nc.declare_dram_parameter("ll", [BPC, C, N], f16, isOutput=False)
    ha_d = nc.declare_dram_parameter("ha", [BPC, C, N], f16, isOutput=False)
    qwT_d = nc.declare_dram_parameter("qwT", [3, 128, 1024], f16, isOutput=False)
    kwT_d = nc.declare_dram_parameter("kwT", [3, 128, 1024], f16, isOutput=False)
    vwT_d = nc.declare_dram_parameter("vwT", [3, 128, 384], f16, isOutput=False)
    pwT_d = nc.declare_dram_parameter("pwT", [4, 128, 384], f16, isOutput=False)
    qb_d = nc.declare_dram_parameter("qb", [128, 8], f32, isOutput=False)
    kb_d = nc.declare_dram_parameter("kb", [128, 8], f32, isOutput=False)
    vb_d = nc.declare_dram_parameter("vb", [1, 384], f16, isOutput=False)
    pb_d = nc.declare_dram_parameter("pb", [128, 3], f32, isOutput=False)
    expb_d = nc.declare_dram_parameter("expb", [NH, N, N], f16, isOutput=False)
    vinit_d = nc.declare_dram_parameter("vinit", [128, 640], f16, isOutput=False)
    out_d = nc.declare_dram_parameter("out", [BPC, C, N], f32, isOutput=True)

    with tile.TileContext(nc) as tc:
        with (
            tc.tile_pool(name="const", bufs=1) as cp,
            tc.tile_pool(name="persist", bufs=1) as pp,
            tc.tile_pool(name="dram", bufs=1, space="DRAM") as dp,
        ):
            # ---- load constants ----
            qwT_sb = [cp.tile([128, 1024], f16, tag=f"qwT{t}", name=f"qwT{t}") for t in range(3)]
            kwT_sb = [cp.tile([128, 1024], f16, tag=f"kwT{t}", name=f"kwT{t}") for t in range(3)]
            vwT_sb = [cp.tile([128, 384], f16, tag=f"vwT{t}", name=f"vwT{t}") for t in range(3)]
            pwT_sb = [cp.tile([128, 384], f16, tag=f"pwT{p}", name=f"pwT{p}") for p in range(4)]
            for t in range(3):
                nc.sync.dma_start(qwT_sb[t][:], qwT_d[t])
                nc.sync.dma_start(kwT_sb[t][:], kwT_d[t])
                nc.sync.dma_start(vwT_sb[t][:], vwT_d[t])
            for p in range(4):
                nc.sync.dma_start(pwT_sb[p][:], pwT_d[p])
            qb_sb = cp.tile([128, 8], f32, tag="qb", name="qb")
            kb_sb = cp.tile([128, 8], f32, tag="kb", name="kb")
            vb_sb = cp.tile([1, 384], f16, tag="vb", name="vb")
            pb_sb = cp.tile([128, 3], f32, tag="pb", name="pb")
            nc.sync.dma_start(qb_sb[:], qb_d[:])
            nc.sync.dma_start(kb_sb[:], kb_d[:])
            nc.sync.dma_start(vb_sb[:], vb_d[:])
            nc.sync.dma_start(pb_sb[:], pb_d[:])
            ones128 = cp.tile([1, 128], f16, tag="ones128", name="ones128")
            nc.gpsimd.memset(ones128[:], 1.0)

            # ---- persistent activation tiles ----
            q_sb = [[pp.tile([128, N], f16, tag=f"q{b}_{h}", name=f"q{b}_{h}")
                     for h in range(NH)] for b in range(BPC)]
            k_sb = [[pp.tile([128, N], f16, tag=f"k{b}_{h}", name=f"k{b}_{h}")
                     for h in range(NH)] for b in range(BPC)]
            vT_sb = [[pp.tile([128, 640], f16, tag=f"vT{b}_{m}", name=f"vT{b}_{m}")
                      for m in range(NMT)] for b in range(BPC)]
            ounT = [pp.tile([65, NH, N], f16, tag=f"ounT{b}", name=f"ounT{b}")
                    for b in range(BPC)]
            onorm = [[pp.tile([128, N], f16, tag=f"onorm{b}_{p}", name=f"onorm{b}_{p}")
                      for p in range(4)] for b in range(BPC)]
            s_all = [pp.tile([16, NP], f16, tag=f"s{b}", name=f"s{b}") for b in range(BPC)]
            r_all = [pp.tile([16, NP], f32, tag=f"r{b}", name=f"r{b}") for b in range(BPC)]
            r16 = [pp.tile([16, NP], f16, tag=f"r16{b}", name=f"r16{b}")
                   for b in range(BPC)]
            bc_all = [pp.tile([48, NH * N], f16, tag=f"bc{b}", name=f"bc{b}")
                      for b in range(BPC)]
            sg_dram = [dp.tile([16, NP], f16, tag=f"sg{b}", name=f"sg{b}")
                       for b in range(BPC)]
            r_dram = [dp.tile([16, NP], f16, tag=f"rd{b}", name=f"rd{b}")
                      for b in range(BPC)]

            # one-time layout init: vT 80-blocks (zeros + ones col @64) via DMA
            # const; onorm pad rows zeroed (32-aligned; data rows rewritten later)
            for b in range(BPC):
                for m in range(NMT):
                    nc.gpsimd.dma_start(vT_sb[b][m][:], vinit_d[:])
                for p in range(4):
                    nc.gpsimd.memset(onorm[b][p][32:64, :], 0.0)
                    nc.gpsimd.memset(onorm[b][p][96:128, :], 0.0)

            # ---- phase A: projections ----
            with (
                tc.tile_pool(name="actA", bufs=1) as apool,
                tc.tile_pool(name="psA", bufs=2, space="PSUM") as psA,
            ):
                ll_sb = [[apool.tile([128, N], f16, tag=f"ll{b}_{t}", name=f"ll{b}_{t}")
                          for t in range(3)] for b in range(BPC)]
                ha_sb = [[apool.tile([128, N], f16, tag=f"ha{b}_{t}", name=f"ha{b}_{t}")
                          for t in range(3)] for b in range(BPC)]
                for b in range(BPC):
                    for t in range(3):
                        nc.sync.dma_start(ll_sb[b][t][:], ll_d[b, 128 * t:128 * (t + 1), :])
                        nc.sync.dma_start(ha_sb[b][t][:], ha_d[b, 128 * t:128 * (t + 1), :])
                    # q / k projections -> duplicated per-head tiles (rows 0-63 == 64-127)
                    for (wt, bt, src_, dst) in (
                        (qwT_sb, qb_sb, ll_sb[b], q_sb[b]),
                        (kwT_sb, kb_sb, ha_sb[b], k_sb[b]),
                    ):
                        for h in range(NH):
                            ps = psA.tile([128, 1024], f32, tag="qk", name="psqk")
                            for nch in range(2):
                                for t in range(3):
                                    nc.tensor.matmul(
                                        ps[:, 512 * nch:512 * nch + NP],
                                        wt[t][:, 128 * h:128 * (h + 1)],
                                        src_[t][:, NP * nch:NP * (nch + 1)],
                                        start=(t == 0),
                                        stop=(t == 2),
                                    )
                            nc.scalar.activation(
                                dst[h].rearrange("p (c n) -> p c n", c=2),
                                ps.rearrange("p (c n) -> p c n", n=512)[:, :, 0:NP],
                                AF.Identity, bias=bt[:, h:h + 1],
                            )
                    # vT projection -> dense 48-blocks, strided copy to 80-blocks
                    for mi, (off, msz) in enumerate(MTS):
                        ps = psA.tile([128, 384], f32, tag="vt", name="psvt")
                        nc.tensor.matmul(ps[0:msz, :], ones128[:, 0:msz], vb_sb[:],
                                         start=True, stop=False)
                        for t in range(3):
                            nc.tensor.matmul(
                                ps[0:msz, :],
                                ha_sb[b][t][:, off:off + msz],
                                vwT_sb[t][:],
                                start=False,
                                stop=(t == 2),
                            )
                        nc.vector.tensor_copy(
                            vT_sb[b][mi].rearrange("p (h c) -> p h c", c=80)[0:msz, :, 0:48],
                            ps.rearrange("p (h c) -> p h c", c=48)[0:msz],
                        )

            # ---- phase B: attention ----
            with (
                tc.tile_pool(name="ebp", bufs=3) as ebp,
                tc.tile_pool(name="etp", bufs=2) as etp,
                tc.tile_pool(name="psqk", bufs=2, space="PSUM") as psqk,
                tc.tile_pool(name="psav", bufs=2, space="PSUM") as psav,
            ):
                eb_tiles = {}

                def attend(h, b):
                    eb_sb = eb_tiles[h]
                    av = psav.tile([65, 1024], f32, tag="av", name="avt")
                    for mi, (off, msz) in enumerate(MTS):
                        eT = etp.tile([128, N], f16, tag="eT", bufs=4, name="eTt")
                        qk = psqk.tile([128, 1024], f32, tag="qk", name="qkt")
                        for nch in range(2):
                            nc.tensor.matmul(
                                qk[0:msz, 512 * nch:512 * nch + NP],
                                k_sb[b][h][:, off:off + msz],
                                q_sb[b][h][:, NP * nch:NP * (nch + 1)],
                                start=True, stop=True,
                            )
                        nc.scalar.activation(
                            eT[0:msz].rearrange("p (c n) -> p c n", c=2),
                            qk[0:msz].rearrange("p (c n) -> p c n", n=512)[:, :, 0:NP],
                            AF.Exp)
                        # exp_bias multiply: split DVE / GPSIMD by m-tile
                        eng = nc.gpsimd if mi in (1, 4) else nc.vector
                        eng.tensor_tensor(
                            eT[0:msz, :], eT[0:msz, :], eb_sb[0:msz, mi, :], MUL)
                        for nch in range(2):
                            nc.tensor.matmul(
                                av[:, 512 * nch:512 * nch + NP],
                                vT_sb[b][mi][0:msz, 80 * h:80 * h + 65],
                                eT[0:msz, NP * nch:NP * (nch + 1)],
                                start=(mi == 0), stop=(mi == NMT - 1),
                            )
                    # evacuate out_unT + s row (f16)
                    nc.vector.tensor_copy(
                        ounT[b][:, h, :].rearrange("p (c n) -> p c n", c=2),
                        av.rearrange("p (c n) -> p c n", n=512)[:, :, 0:NP],
                    )

                def tail(b):
                    # batched softmax denominators + normalization
                    nc.sync.dma_start(
                        sg_dram[b].rearrange("p n -> () (p n)").rearrange(
                            "() (h n) -> () h n", n=N),
                        ounT[b][64:65, :, :],
                    )
                    nc.sync.dma_start(s_all[b][:], sg_dram[b][:])
                    nc.vector.reciprocal(r_all[b][:], s_all[b][:])
                    nc.vector.tensor_copy(r16[b][:], r_all[b][:])
                    nc.sync.dma_start(r_dram[b][:], r16[b][:])
                    nc.sync.dma_start(
                        bc_all[b][:],
                        r_dram[b].tensor.ap().rearrange(
                            "p n -> () (p n)").to_broadcast((48, NH * N)),
                    )
                    for h in range(NH):
                        prr, hpp = divmod(h, 2)
                        nc.vector.tensor_tensor(
                            onorm[b][prr][64 * hpp:64 * hpp + 48, :],
                            ounT[b][0:48, h, :],
                            bc_all[b][:, N * h:N * (h + 1)],
                            MUL,
                        )

                def proj(b):
                    for o in range(3):
                        ps = psav.tile([128, 1024], f32, tag="av", name="psy")
                        for nch in range(2):
                            for p in range(4):
                                nc.tensor.matmul(
                                    ps[:, 512 * nch:512 * nch + NP],
                                    pwT_sb[p][:, 128 * o:128 * (o + 1)],
                                    onorm[b][p][:, NP * nch:NP * (nch + 1)],
                                    start=(p == 0), stop=(p == 3),
                                )
                        y_sb = ypool.tile([128, N], f32, tag="y", name="ysb")
                        nc.scalar.activation(
                            y_sb.rearrange("p (c n) -> p c n", c=2),
                            ps.rearrange("p (c n) -> p c n", n=512)[:, :, 0:NP],
                            AF.Identity, bias=pb_sb[:, o:o + 1])
                        nc.sync.dma_start(
                            out_d[b, 128 * o:128 * (o + 1), :], y_sb[:])

                with tc.tile_pool(name="yp", bufs=3) as ypool:
                    # b=1 lags one head so b=0's tail overlaps b=1's last heads
                    sched = []
                    for h in range(NH):
                        sched.append((h, 0))
                        if h >= 1:
                            sched.append((h - 1, 1))
                    sched.append((NH - 1, 1))
                    for (h, b) in sched:
                        if b == 0 and h not in eb_tiles:
                            eb_sb = ebp.tile([128, NMT, N], f16, tag="eb", name="ebt")
                            for mi, (off, msz) in enumerate(MTS):
                                nc.sync.dma_start(eb_sb[0:msz, mi, :],
                                                  expb_d[h, off:off + msz, :])
                            eb_tiles[h] = eb_sb
                        attend(h, b)
                        if h == NH - 1:
                            tail(b)
                            proj(b)

    nc.finalize()
    return nc


def _prep_consts(q_w, q_b, kv_w, kv_b, proj_w, proj_b, attn_biases, bias_idxs):
    f16 = np.float16
    qw = (q_w * SCALE).astype(np.float32)
    qb = (q_b * SCALE).astype(np.float32)
    kw = kv_w[:C] * 0.5
    kb = kv_b[:C] * 0.5
    vw = kv_w[C:]
    vb = kv_b[C:]

    def pad64(w2, b1):  # [384(o), 384(c)] -> [512, 384] / [512]
        wp = np.zeros((512, C), np.float32)
        bp = np.zeros((512,), np.float32)
        for h in range(NH):
            wp[64 * h:64 * h + HD] = w2[HD * h:HD * (h + 1)]
            bp[64 * h:64 * h + HD] = b1[HD * h:HD * (h + 1)]
        return wp, bp

    qwp, qbp = pad64(qw, qb)
    kwp, kbp = pad64(kw, kb)

    def dup(w, b1):  # [512, C] -> [1024, C]: per-head 64-block duplicated
        wd = np.zeros((1024, C), np.float32)
        bd = np.zeros((1024,), np.float32)
        for h in range(NH):
            for r in range(2):
                wd[128 * h + 64 * r:128 * h + 64 * r + 64] = w[64 * h:64 * h + 64]
                bd[128 * h + 64 * r:128 * h + 64 * r + 64] = b1[64 * h:64 * h + 64]
        return wd, bd

    qwd, qbd = dup(qwp, qbp)
    kwd, kbd = dup(kwp, kbp)
    qwT = np.ascontiguousarray(qwd.T.reshape(3, 128, 1024)).astype(f16)
    kwT = np.ascontiguousarray(kwd.T.reshape(3, 128, 1024)).astype(f16)
    vwT = np.ascontiguousarray(vw.T.reshape(3, 128, C)).astype(f16)

    # proj weights in onorm pair-tile layout: pair p row j -> channel
    pwT = np.zeros((4, 128, C), np.float32)
    for p in range(4):
        pwT[p, 0:HD] = proj_w[:, 96 * p:96 * p + HD].T
        pwT[p, 64:64 + HD] = proj_w[:, 96 * p + HD:96 * p + 96].T
    pwT = pwT.astype(f16)

    qb_h = np.ascontiguousarray(qbd.reshape(8, 128).T).astype(np.float32)
    kb_h = np.ascontiguousarray(kbd.reshape(8, 128).T).astype(np.float32)
    pb_h = np.ascontiguousarray(proj_b.reshape(3, 128).T).astype(np.float32)
    vb_h = vb.reshape(1, C).astype(f16)

    expb = np.ascontiguousarray(np.exp(attn_biases[:, bias_idxs]).astype(f16))

    vinit = np.zeros((128, 640), f16)
    vinit[:, 64::80] = 1.0

    return dict(qwT=qwT, kwT=kwT, vwT=vwT, pwT=pwT, qb=qb_h, kb=kb_h,
                vb=vb_h, pb=pb_h, expb=expb, vinit=vinit)


def kernel(ll, high_attn, q_w, q_b, kv_w, kv_b, proj_w, proj_b,
           attn_biases, bias_idxs):
    from concourse.bass_utils import run_bass_kernel_spmd

    global LAST_RESULTS
    ll = np.asarray(ll)
    high_attn = np.asarray(high_attn)

    if "nc" not in _CACHE:
        _CACHE["nc"] = _build_nc()
    nc = _CACHE["nc"]

    consts = _prep_consts(
        np.asarray(q_w), np.asarray(q_b), np.asarray(kv_w), np.asarray(kv_b),
        np.asarray(proj_w), np.asarray(proj_b), np.asarray(attn_biases),
        np.asarray(bias_idxs),
    )

    ll16 = ll.reshape(B, C, N).astype(np.float16)
    ha16 = high_attn.reshape(B, C, N).astype(np.float16)

    in_maps = []
    for i in range(NCORES):
        m = {"ll": ll16[BPC * i:BPC * (i + 1)], "ha": ha16[BPC * i:BPC * (i + 1)]}
        m.update(consts)
        in_maps.append(m)

    res = run_bass_kernel_spmd(nc, in_maps, core_ids=list(range(NCORES)),
                               trace=TRACE)
    LAST_RESULTS = {"exec_time_ns": res.exec_time_ns,
                    "scope_times": res.per_core_scope_times}

    out = np.empty((B, C, N), np.float32)
    for i in range(NCORES):
        out[BPC * i:BPC * (i + 1)] = res.results[i]["out"]
    return out.reshape(B, C, RES, RES)


# revision 11
# speedup vs baseline: 1.1469x; 1.1018x over previous
"""Trainium2 Bass kernel for nn_LowFreqCrossAttn (dense transformer cross-attention).

Data-parallel over batch: 16 batches -> 8 NeuronCores, 2 batches/core.
Weights / attention-bias tables replicated.

Per-core dataflow (all matmuls fp16 x fp16 -> f32 PSUM):
  A) q = (s*Wq) @ ll, k = (0.5*Wk) @ ha   (head rows duplicated to K=128:
     K<128 matmul streams never warm the PE HAM clock gate -> half clock)
     vT = ha^T @ WvT (+bias row)          (token-major, dense 48-col head
     blocks, re-laid to 80-col blocks with a ones col @64)
  B) per (head, batch): logitsT = k_h^T q_h  (m on partitions, n free)
     e = exp(logitsT) * exp_bias^T  (ACT exp -> f16, DVE/GPS mult; no
     max-subtraction: |logits + bias| <= 1.1 for this model)
     out_unT[d, n] (+ s row @ partition 64) = vT_slice^T @ e  (PSUM accum
     over m-tiles; QK + AV psum tiles span 2 banks so exp/evac run as one
     fat strided op per (h, b, m) instead of per chunk)
  B-tail, per batch) s rows -> DRAM bounce -> [16, 392] -> one batched DVE
     reciprocal -> DRAM bounce -> [1, 6272] -> gpsimd partition_broadcast
     out_norm = out_unT * (1/s)  (pair tiles, c-major, f16)
  C) y = WpT^T @ out_norm + b  (channel-major f32 out)
"""

import numpy as np

B = 16
C = 384
RES = 28
N = 784
NH = 8
HD = 48
NP = 392            # n-chunk (half of N; fits one PSUM bank in f32)
NCORES = 8
BPC = 2             # batches per core
SCALE = HD ** -0.5
# m-tiles: 6 x 128 + 1 x 16 (K=128 keeps the PE HAM warm)
MTS = [(128 * i, 128) for i in range(6)] + [(768, 16)]
NMT = len(MTS)

TRACE = False       # set True to capture an NTFF trace on core 0
LAST_RESULTS = {}   # exec_time_ns etc. from the last run (when TRACE)

_CACHE = {}


def _build_nc():
    import concourse.bacc as bacc
    import concourse.mybir as mybir
    import concourse.tile as tile

    f16 = mybir.dt.float16
    f32 = mybir.dt.float32
    AF = mybir.ActivationFunctionType
    MUL = mybir.AluOpType.mult

    nc = bacc.Bacc("TRN2", target_bir_lowering=False, debug=False)

    ll_d = nc.declare_dram_parameter("ll", [BPC, C, N], f16, isOutput=False)
    ha_d = nc.declare_dram_parameter("ha", [BPC, C, N], f16, isOutput=False)
    qwT_d = nc.declare_dram_parameter("qwT", [3, 128, 512], f16, isOutput=False)
    kwT_d = nc.declare_dram_parameter("kwT", [3, 128, 512], f16, isOutput=False)
    vwT_d = nc.declare_dram_parameter("vwT", [3, 128, 384], f16, isOutput=False)
    pwT_d = nc.declare_dram_parameter("pwT", [4, 128, 384], f16, isOutput=False)
    qb_d = nc.declare_dram_parameter("qb", [128, 4], f32, isOutput=False)
    kb_d = nc.declare_dram_parameter("kb", [128, 4], f32, isOutput=False)
    vb_d = nc.declare_dram_parameter("vb", [1, 384], f16, isOutput=False)
    pb_d = nc.declare_dram_parameter("pb", [128, 3], f32, isOutput=False)
    expb_d = nc.declare_dram_parameter("expb", [NH, N, N], f16, isOutput=False)
    vinit_d = nc.declare_dram_parameter("vinit", [128, 640], f16, isOutput=False)
    out_d = nc.declare_dram_parameter("out", [BPC, C, N], f32, isOutput=True)

    with tile.TileContext(nc) as tc:
        with (
            tc.tile_pool(name="const", bufs=1) as cp,
            tc.tile_pool(name="persist", bufs=1) as pp,
            tc.tile_pool(name="dram", bufs=1, space="DRAM") as dp,
        ):
            # ---- load constants ----
            qwT_sb = [cp.tile([128, 512], f16, tag=f"qwT{t}", name=f"qwT{t}") for t in range(3)]
            kwT_sb = [cp.tile([128, 512], f16, tag=f"kwT{t}", name=f"kwT{t}") for t in range(3)]
            vwT_sb = [cp.tile([128, 384], f16, tag=f"vwT{t}", name=f"vwT{t}") for t in range(3)]
            pwT_sb = [cp.tile([128, 384], f16, tag=f"pwT{p}", name=f"pwT{p}") for p in range(4)]
            for t in range(3):
                nc.sync.dma_start(qwT_sb[t][:], qwT_d[t])
                nc.sync.dma_start(kwT_sb[t][:], kwT_d[t])
                nc.sync.dma_start(vwT_sb[t][:], vwT_d[t])
            for p in range(4):
                nc.sync.dma_start(pwT_sb[p][:], pwT_d[p])
            qb_sb = cp.tile([128, 4], f32, tag="qb", name="qb")
            kb_sb = cp.tile([128, 4], f32, tag="kb", name="kb")
            vb_sb = cp.tile([1, 384], f16, tag="vb", name="vb")
            pb_sb = cp.tile([128, 3], f32, tag="pb", name="pb")
            nc.sync.dma_start(qb_sb[:], qb_d[:])
            nc.sync.dma_start(kb_sb[:], kb_d[:])
            nc.sync.dma_start(vb_sb[:], vb_d[:])
            nc.sync.dma_start(pb_sb[:], pb_d[:])
            ones128 = cp.tile([1, 128], f16, tag="ones128", name="ones128")
            nc.gpsimd.memset(ones128[:], 1.0)

            # ---- persistent activation tiles ----
            q_sb = [[pp.tile([128, N], f16, tag=f"q{b}_{h}", name=f"q{b}_{h}")
                     for h in range(NH)] for b in range(BPC)]
            k_sb = [[pp.tile([128, N], f16, tag=f"k{b}_{h}", name=f"k{b}_{h}")
                     for h in range(NH)] for b in range(BPC)]
            vT_sb = [[pp.tile([128, 640], f16, tag=f"vT{b}_{m}", name=f"vT{b}_{m}")
                      for m in range(NMT)] for b in range(BPC)]
            ounT = [pp.tile([65, NH, N], f16, tag=f"ounT{b}", name=f"ounT{b}")
                    for b in range(BPC)]
            onorm = [[pp.tile([128, N], f16, tag=f"onorm{b}_{p}", name=f"onorm{b}_{p}")
                      for p in range(4)] for b in range(BPC)]
            s_all = [pp.tile([16, NP], f16, tag=f"s{b}", name=f"s{b}") for b in range(BPC)]
            r_all = [pp.tile([16, NP], f32, tag=f"r{b}", name=f"r{b}") for b in range(BPC)]
            r16 = [pp.tile([16, NP], f16, tag=f"r16{b}", name=f"r16{b}")
                   for b in range(BPC)]
            bc_all = [pp.tile([48, NH * N], f16, tag=f"bc{b}", name=f"bc{b}")
                      for b in range(BPC)]
            sg_dram = [dp.tile([16, NP], f16, tag=f"sg{b}", name=f"sg{b}")
                       for b in range(BPC)]
            r_dram = [dp.tile([16, NP], f16, tag=f"rd{b}", name=f"rd{b}")
                      for b in range(BPC)]

            # one-time layout init: vT 80-blocks (zeros + ones col @64) via DMA
            # const; onorm pad rows zeroed (32-aligned; data rows rewritten later)
            for b in range(BPC):
                for m in range(NMT):
                    nc.gpsimd.dma_start(vT_sb[b][m][:], vinit_d[:])
                for p in range(4):
                    nc.gpsimd.memset(onorm[b][p][32:64, :], 0.0)
                    nc.gpsimd.memset(onorm[b][p][96:128, :], 0.0)

            # ---- phase A: projections ----
            with (
                tc.tile_pool(name="actA", bufs=1) as apool,
                tc.tile_pool(name="psA", bufs=2, space="PSUM") as psA,
            ):
                ll_sb = [[apool.tile([128, N], f16, tag=f"ll{b}_{t}", name=f"ll{b}_{t}")
                          for t in range(3)] for b in range(BPC)]
                ha_sb = [[apool.tile([128, N], f16, tag=f"ha{b}_{t}", name=f"ha{b}_{t}")
                          for t in range(3)] for b in range(BPC)]
                for b in range(BPC):
                    for t in range(3):
                        nc.sync.dma_start(ll_sb[b][t][:], ll_d[b, 128 * t:128 * (t + 1), :])
                        nc.sync.dma_start(ha_sb[b][t][:], ha_d[b, 128 * t:128 * (t + 1), :])
                    # q / k projections -> head-pair tiles in q_sb[2p], then
                    # duplicate rows to build per-head K=128 tiles via DMAs
                    for (wt, bt, src_, dst) in (
                        (qwT_sb, qb_sb, ll_sb[b], q_sb[b]),
                        (kwT_sb, kb_sb, ha_sb[b], k_sb[b]),
                    ):
                        for p in range(4):
                            ps = psA.tile([128, 1024], f32, tag="qk", name="psqk")
                            for nch in range(2):
                                for t in range(3):
                                    nc.tensor.matmul(
                                        ps[:, 512 * nch:512 * nch + NP],
                                        wt[t][:, 128 * p:128 * (p + 1)],
                                        src_[t][:, NP * nch:NP * (nch + 1)],
                                        start=(t == 0),
                                        stop=(t == 2),
                                    )
                            nc.vector.tensor_scalar_add(
                                dst[2 * p].rearrange("p (c n) -> p c n", c=2),
                                ps.rearrange("p (c n) -> p c n", n=512)[:, :, 0:NP],
                                bt[:, p:p + 1],
                            )
                            nc.sync.dma_start(dst[2 * p + 1][0:64, :], dst[2 * p][64:128, :])
                            nc.sync.dma_start(dst[2 * p + 1][64:128, :], dst[2 * p][64:128, :])
                            nc.sync.dma_start(dst[2 * p][64:128, :], dst[2 * p][0:64, :])
                    # vT projection -> dense 48-blocks, strided copy to 80-blocks
                    for mi, (off, msz) in enumerate(MTS):
                        ps = psA.tile([128, 384], f32, tag="vt", name="psvt")
                        nc.tensor.matmul(ps[0:msz, :], ones128[:, 0:msz], vb_sb[:],
                                         start=True, stop=False)
                        for t in range(3):
                            nc.tensor.matmul(
                                ps[0:msz, :],
                                ha_sb[b][t][:, off:off + msz],
                                vwT_sb[t][:],
                                start=False,
                                stop=(t == 2),
                            )
                        nc.vector.tensor_copy(
                            vT_sb[b][mi].rearrange("p (h c) -> p h c", c=80)[0:msz, :, 0:48],
                            ps.rearrange("p (h c) -> p h c", c=48)[0:msz],
                        )

            # ---- phase B: attention ----
            with (
                tc.tile_pool(name="ebp", bufs=3) as ebp,
                tc.tile_pool(name="etp", bufs=2) as etp,
                tc.tile_pool(name="psqk", bufs=2, space="PSUM") as psqk,
                tc.tile_pool(name="psav", bufs=2, space="PSUM") as psav,
            ):
                eb_tiles = {}

                def attend(h, b):
                    eb_sb = eb_tiles[h]
                    av = psav.tile([65, 1024], f32, tag="av", name="avt")
                    for mi, (off, msz) in enumerate(MTS):
                        eT = etp.tile([128, N], f16, tag="eT", bufs=4, name="eTt")
                        qk = psqk.tile([128, 1024], f32, tag="qk", name="qkt")
                        for nch in range(2):
                            nc.tensor.matmul(
                                qk[0:msz, 512 * nch:512 * nch + NP],
                                k_sb[b][h][:, off:off + msz],
                                q_sb[b][h][:, NP * nch:NP * (nch + 1)],
                                start=True, stop=True,
                            )
                        nc.scalar.activation(
                            eT[0:msz].rearrange("p (c n) -> p c n", c=2),
                            qk[0:msz].rearrange("p (c n) -> p c n", n=512)[:, :, 0:NP],
                            AF.Exp)
                        # exp_bias multiply: split DVE / GPSIMD by m-tile
                        eng = nc.gpsimd if mi in (1, 3, 5) else nc.vector
                        eng.tensor_tensor(
                            eT[0:msz, :], eT[0:msz, :], eb_sb[0:msz, mi, :], MUL)
                        for nch in range(2):
                            nc.tensor.matmul(
                                av[:, 512 * nch:512 * nch + NP],
                                vT_sb[b][mi][0:msz, 80 * h:80 * h + 65],
                                eT[0:msz, NP * nch:NP * (nch + 1)],
                                start=(mi == 0), stop=(mi == NMT - 1),
                            )
                    # evacuate out_unT + s row (f16)
                    nc.vector.tensor_copy(
                        ounT[b][:, h, :].rearrange("p (c n) -> p c n", c=2),
                        av.rearrange("p (c n) -> p c n", n=512)[:, :, 0:NP],
                    )

                def tail(b):
                    # batched softmax denominators + normalization
                    nc.sync.dma_start(
                        sg_dram[b].rearrange("p n -> () (p n)").rearrange(
                            "() (h n) -> () h n", n=N),
                        ounT[b][64:65, :, :],
                    )
                    nc.sync.dma_start(s_all[b][:], sg_dram[b][:])
                    nc.vector.reciprocal(r_all[b][:], s_all[b][:])
                    nc.vector.tensor_copy(r16[b][:], r_all[b][:])
                    nc.sync.dma_start(r_dram[b][:], r16[b][:])
                    nc.sync.dma_start(
                        bc_all[b][:],
                        r_dram[b].tensor.ap().rearrange(
                            "p n -> () (p n)").to_broadcast((48, NH * N)),
                    )
                    for h in range(NH):
                        prr, hpp = divmod(h, 2)
                        nc.vector.tensor_tensor(
                            onorm[b][prr][64 * hpp:64 * hpp + 48, :],
                            ounT[b][0:48, h, :],
                            bc_all[b][:, N * h:N * (h + 1)],
                            MUL,
                        )

                def proj(b):
                    for o in range(3):
                        ps = psav.tile([128, 1024], f32, tag="av", name="psy")
                        for nch in range(2):
                            for p in range(4):
                                nc.tensor.matmul(
                                    ps[:, 512 * nch:512 * nch + NP],
                                    pwT_sb[p][:, 128 * o:128 * (o + 1)],
                                    onorm[b][p][:, NP * nch:NP * (nch + 1)],
                                    start=(p == 0), stop=(p == 3),
                                )
                        y_sb = ypool.tile([128, N], f32, tag="y", name="ysb")
                        nc.scalar.activation(
                            y_sb.rearrange("p (c n) -> p c n", c=2),
                            ps.rearrange("p (c n) -> p c n", n=512)[:, :, 0:NP],
                            AF.Identity, bias=pb_sb[:, o:o + 1])
                        nc.sync.dma_start(
                            out_d[b, 128 * o:128 * (o + 1), :], y_sb[:])

                with tc.tile_pool(name="yp", bufs=3) as ypool:
                    # b=1 lags one head so b=0's tail overlaps b=1's last heads
                    sched = []
                    for h in range(NH):
                        sched.append((h, 0))
                        if h >= 1:
                            sched.append((h - 1, 1))
                    sched.append((NH - 1, 1))
                    for (h, b) in sched:
                        if b == 0 and h not in eb_tiles:
                            eb_sb = ebp.tile([128, NMT, N], f16, tag="eb", name="ebt")
                            for mi, (off, msz) in enumerate(MTS):
                                nc.sync.dma_start(eb_sb[0:msz, mi, :],
                                                  expb_d[h, off:off + msz, :])
                            eb_tiles[h] = eb_sb
                        attend(h, b)
                        if h == NH - 1:
                            tail(b)
                    proj(0)
                    proj(1)

    nc.finalize()
    return nc


def _prep_consts(q_w, q_b, kv_w, kv_b, proj_w, proj_b, attn_biases, bias_idxs):
    f16 = np.float16
    qw = (q_w * SCALE).astype(np.float32)
    qb = (q_b * SCALE).astype(np.float32)
    kw = kv_w[:C] * 0.5
    kb = kv_b[:C] * 0.5
    vw = kv_w[C:]
    vb = kv_b[C:]

    def pad64(w2, b1):  # [384(o), 384(c)] -> [512, 384] / [512]
        wp = np.zeros((512, C), np.float32)
        bp = np.zeros((512,), np.float32)
        for h in range(NH):
            wp[64 * h:64 * h + HD] = w2[HD * h:HD * (h + 1)]
            bp[64 * h:64 * h + HD] = b1[HD * h:HD * (h + 1)]
        return wp, bp

    qwp, qbp = pad64(qw, qb)
    kwp, kbp = pad64(kw, kb)
    qwT = np.ascontiguousarray(qwp.T.reshape(3, 128, 512)).astype(f16)
    kwT = np.ascontiguousarray(kwp.T.reshape(3, 128, 512)).astype(f16)
    vwT = np.ascontiguousarray(vw.T.reshape(3, 128, C)).astype(f16)

    # proj weights in onorm pair-tile layout: pair p row j -> channel
    pwT = np.zeros((4, 128, C), np.float32)
    for p in range(4):
        pwT[p, 0:HD] = proj_w[:, 96 * p:96 * p + HD].T
        pwT[p, 64:64 + HD] = proj_w[:, 96 * p + HD:96 * p + 96].T
    pwT = pwT.astype(f16)

    qb_h = np.ascontiguousarray(qbp.reshape(4, 128).T).astype(np.float32)
    kb_h = np.ascontiguousarray(kbp.reshape(4, 128).T).astype(np.float32)
    pb_h = np.ascontiguousarray(proj_b.reshape(3, 128).T).astype(np.float32)
    vb_h = vb.reshape(1, C).astype(f16)

    expb = np.ascontiguousarray(np.exp(attn_biases[:, bias_idxs]).astype(f16))

    vinit = np.zeros((128, 640), f16)
    vinit[:, 64::80] = 1.0

    return dict(qwT=qwT, kwT=kwT, vwT=vwT, pwT=pwT, qb=qb_h, kb=kb_h,
                vb=vb_h, pb=pb_h, expb=expb, vinit=vinit)


def kernel(ll, high_attn, q_w, q_b, kv_w, kv_b, proj_w, proj_b,
           attn_biases, bias_idxs):
    from concourse.bass_utils import run_bass_kernel_spmd

    global LAST_RESULTS
    ll = np.asarray(ll)
    high_attn = np.asarray(high_attn)

    if "nc" not in _CACHE:
        _CACHE["nc"] = _build_nc()
    nc = _CACHE["nc"]

    consts = _prep_consts(
        np.asarray(q_w), np.asarray(q_b), np.asarray(kv_w), np.asarray(kv_b),
        np.asarray(proj_w), np.asarray(proj_b), np.asarray(attn_biases),
        np.asarray(bias_idxs),
    )

    ll16 = ll.reshape(B, C, N).astype(np.float16)
    ha16 = high_attn.reshape(B, C, N).astype(np.float16)

    in_maps = []
    for i in range(NCORES):
        m = {"ll": ll16[BPC * i:BPC * (i + 1)], "ha": ha16[BPC * i:BPC * (i + 1)]}
        m.update(consts)
        in_maps.append(m)

    res = run_bass_kernel_spmd(nc, in_maps, core_ids=list(range(NCORES)),
                               trace=TRACE)
    LAST_RESULTS = {"exec_time_ns": res.exec_time_ns,
                    "scope_times": res.per_core_scope_times}

    out = np.empty((B, C, N), np.float32)
    for i in range(NCORES):
        out[BPC * i:BPC * (i + 1)] = res.results[i]["out"]
    return out.reshape(B, C, RES, RES)


# revision 12
# speedup vs baseline: 1.1586x; 1.0103x over previous
"""Trainium2 Bass kernel for nn_LowFreqCrossAttn (dense transformer cross-attention).

Data-parallel over batch: 16 batches -> 8 NeuronCores, 2 batches/core.
Weights / attention-bias tables replicated.

Per-core dataflow (all matmuls fp16 x fp16 -> f32 PSUM):
  A) q = (s*Wq) @ ll, k = (0.5*Wk) @ ha   (head rows duplicated to K=128:
     K<128 matmul streams never warm the PE HAM clock gate -> half clock)
     vT = ha^T @ WvT (+bias row)          (token-major, dense 48-col head
     blocks, re-laid to 80-col blocks with a ones col @64)
  B) per (head, batch): logitsT = k_h^T q_h  (m on partitions, n free)
     e = exp(logitsT) * exp_bias^T  (ACT exp -> f16, DVE/GPS mult; no
     max-subtraction: |logits + bias| <= 1.1 for this model)
     out_unT[d, n] (+ s row @ partition 64) = vT_slice^T @ e  (PSUM accum
     over m-tiles; QK + AV psum tiles span 2 banks so exp/evac run as one
     fat strided op per (h, b, m) instead of per chunk)
  B-tail, per batch) s rows -> DRAM bounce -> [16, 392] -> one batched DVE
     reciprocal -> DRAM bounce -> [1, 6272] -> gpsimd partition_broadcast
     out_norm = out_unT * (1/s)  (pair tiles, c-major, f16)
  C) y = WpT^T @ out_norm + b  (channel-major f32 out)
"""

import numpy as np

B = 16
C = 384
RES = 28
N = 784
NH = 8
HD = 48
NP = 392            # n-chunk (half of N; fits one PSUM bank in f32)
NCORES = 8
BPC = 2             # batches per core
SCALE = HD ** -0.5
# m-tiles: 6 x 128 + 1 x 16 (K=128 keeps the PE HAM warm)
MTS = [(128 * i, 128) for i in range(6)] + [(768, 16)]
NMT = len(MTS)

TRACE = False       # set True to capture an NTFF trace on core 0
LAST_RESULTS = {}   # exec_time_ns etc. from the last run (when TRACE)

_CACHE = {}


def _build_nc():
    import concourse.bacc as bacc
    import concourse.mybir as mybir
    import concourse.tile as tile

    f16 = mybir.dt.float16
    f32 = mybir.dt.float32
    AF = mybir.ActivationFunctionType
    MUL = mybir.AluOpType.mult

    nc = bacc.Bacc("TRN2", target_bir_lowering=False, debug=False)

    ll_d = nc.declare_dram_parameter("ll", [BPC, C, N], f16, isOutput=False)
    ha_d = nc.declare_dram_parameter("ha", [BPC, C, N], f16, isOutput=False)
    qwT_d = nc.declare_dram_parameter("qwT", [3, 128, 512], f16, isOutput=False)
    kwT_d = nc.declare_dram_parameter("kwT", [3, 128, 512], f16, isOutput=False)
    vwT_d = nc.declare_dram_parameter("vwT", [3, 128, 384], f16, isOutput=False)
    pwT_d = nc.declare_dram_parameter("pwT", [4, 128, 384], f16, isOutput=False)
    qb_d = nc.declare_dram_parameter("qb", [128, 4], f32, isOutput=False)
    kb_d = nc.declare_dram_parameter("kb", [128, 4], f32, isOutput=False)
    vb_d = nc.declare_dram_parameter("vb", [1, 384], f16, isOutput=False)
    pb_d = nc.declare_dram_parameter("pb", [128, 3], f32, isOutput=False)
    expb_d = nc.declare_dram_parameter("expb", [NH, N, N], f16, isOutput=False)
    vinit_d = nc.declare_dram_parameter("vinit", [128, 640], f16, isOutput=False)
    out_d = nc.declare_dram_parameter("out", [BPC, C, N], f32, isOutput=True)

    with tile.TileContext(nc) as tc:
        with (
            tc.tile_pool(name="const", bufs=1) as cp,
            tc.tile_pool(name="persist", bufs=1) as pp,
            tc.tile_pool(name="dram", bufs=1, space="DRAM") as dp,
        ):
            # ---- load constants ----
            qwT_sb = [cp.tile([128, 512], f16, tag=f"qwT{t}", name=f"qwT{t}") for t in range(3)]
            kwT_sb = [cp.tile([128, 512], f16, tag=f"kwT{t}", name=f"kwT{t}") for t in range(3)]
            vwT_sb = [cp.tile([128, 384], f16, tag=f"vwT{t}", name=f"vwT{t}") for t in range(3)]
            pwT_sb = [cp.tile([128, 384], f16, tag=f"pwT{p}", name=f"pwT{p}") for p in range(4)]
            for t in range(3):
                nc.sync.dma_start(qwT_sb[t][:], qwT_d[t])
                nc.sync.dma_start(kwT_sb[t][:], kwT_d[t])
                nc.sync.dma_start(vwT_sb[t][:], vwT_d[t])
            for p in range(4):
                nc.sync.dma_start(pwT_sb[p][:], pwT_d[p])
            qb_sb = cp.tile([128, 4], f32, tag="qb", name="qb")
            kb_sb = cp.tile([128, 4], f32, tag="kb", name="kb")
            vb_sb = cp.tile([1, 384], f16, tag="vb", name="vb")
            pb_sb = cp.tile([128, 3], f32, tag="pb", name="pb")
            nc.sync.dma_start(qb_sb[:], qb_d[:])
            nc.sync.dma_start(kb_sb[:], kb_d[:])
            nc.sync.dma_start(vb_sb[:], vb_d[:])
            nc.sync.dma_start(pb_sb[:], pb_d[:])
            ones128 = cp.tile([1, 128], f16, tag="ones128", name="ones128")
            nc.gpsimd.memset(ones128[:], 1.0)

            # ---- persistent activation tiles ----
            q_sb = [[pp.tile([128, N], f16, tag=f"q{b}_{h}", name=f"q{b}_{h}")
                     for h in range(NH)] for b in range(BPC)]
            k_sb = [[pp.tile([128, N], f16, tag=f"k{b}_{h}", name=f"k{b}_{h}")
                     for h in range(NH)] for b in range(BPC)]
            vT_sb = [[pp.tile([128, 640], f16, tag=f"vT{b}_{m}", name=f"vT{b}_{m}")
                      for m in range(NMT)] for b in range(BPC)]
            ounT = [pp.tile([65, NH, N], f16, tag=f"ounT{b}", name=f"ounT{b}")
                    for b in range(BPC)]
            onorm = [[pp.tile([128, N], f16, tag=f"onorm{b}_{p}", name=f"onorm{b}_{p}")
                      for p in range(4)] for b in range(BPC)]
            s_all = [pp.tile([16, NP], f16, tag=f"s{b}", name=f"s{b}") for b in range(BPC)]
            r_all = [pp.tile([16, NP], f32, tag=f"r{b}", name=f"r{b}") for b in range(BPC)]
            r16 = [pp.tile([16, NP], f16, tag=f"r16{b}", name=f"r16{b}")
                   for b in range(BPC)]
            bc_all = [pp.tile([48, NH * N], f16, tag=f"bc{b}", name=f"bc{b}")
                      for b in range(BPC)]
            sg_dram = [dp.tile([16, NP], f16, tag=f"sg{b}", name=f"sg{b}")
                       for b in range(BPC)]
            r_dram = [dp.tile([16, NP], f16, tag=f"rd{b}", name=f"rd{b}")
                      for b in range(BPC)]

            # one-time layout init: vT 80-blocks (zeros + ones col @64) via DMA
            # const; onorm pad rows zeroed (32-aligned; data rows rewritten later)
            for b in range(BPC):
                for m in range(NMT):
                    nc.gpsimd.dma_start(vT_sb[b][m][:], vinit_d[:])
                for p in range(4):
                    nc.gpsimd.memset(onorm[b][p][32:64, :], 0.0)
                    nc.gpsimd.memset(onorm[b][p][96:128, :], 0.0)

            # ---- phase A: projections ----
            with (
                tc.tile_pool(name="actA", bufs=1) as apool,
                tc.tile_pool(name="psA", bufs=2, space="PSUM") as psA,
            ):
                ll_sb = [[apool.tile([128, N], f16, tag=f"ll{b}_{t}", name=f"ll{b}_{t}")
                          for t in range(3)] for b in range(BPC)]
                ha_sb = [[apool.tile([128, N], f16, tag=f"ha{b}_{t}", name=f"ha{b}_{t}")
                          for t in range(3)] for b in range(BPC)]
                for b in range(BPC):
                    for t in range(3):
                        nc.sync.dma_start(ll_sb[b][t][:], ll_d[b, 128 * t:128 * (t + 1), :])
                        nc.sync.dma_start(ha_sb[b][t][:], ha_d[b, 128 * t:128 * (t + 1), :])
                    # q / k projections -> head-pair tiles in q_sb[2p], then
                    # duplicate rows to build per-head K=128 tiles via DMAs
                    for (wt, bt, src_, dst) in (
                        (qwT_sb, qb_sb, ll_sb[b], q_sb[b]),
                        (kwT_sb, kb_sb, ha_sb[b], k_sb[b]),
                    ):
                        for p in range(4):
                            ps = psA.tile([128, 1024], f32, tag="qk", name="psqk")
                            for nch in range(2):
                                for t in range(3):
                                    nc.tensor.matmul(
                                        ps[:, 512 * nch:512 * nch + NP],
                                        wt[t][:, 128 * p:128 * (p + 1)],
                                        src_[t][:, NP * nch:NP * (nch + 1)],
                                        start=(t == 0),
                                        stop=(t == 2),
                                    )
                            nc.scalar.activation(
                                dst[2 * p].rearrange("p (c n) -> p c n", c=2),
                                ps.rearrange("p (c n) -> p c n", n=512)[:, :, 0:NP],
                                AF.Identity, bias=bt[:, p:p + 1],
                            )
                            nc.gpsimd.dma_start(dst[2 * p + 1][0:64, :], dst[2 * p][64:128, :])
                            nc.gpsimd.dma_start(dst[2 * p + 1][64:128, :], dst[2 * p][64:128, :])
                            nc.gpsimd.dma_start(dst[2 * p][64:128, :], dst[2 * p][0:64, :])
                    # vT projection -> dense 48-blocks, strided copy to 80-blocks
                    for mi, (off, msz) in enumerate(MTS):
                        ps = psA.tile([128, 384], f32, tag="vt", name="psvt")
                        nc.tensor.matmul(ps[0:msz, :], ones128[:, 0:msz], vb_sb[:],
                                         start=True, stop=False)
                        for t in range(3):
                            nc.tensor.matmul(
                                ps[0:msz, :],
                                ha_sb[b][t][:, off:off + msz],
                                vwT_sb[t][:],
                                start=False,
                                stop=(t == 2),
                            )
                        nc.scalar.activation(
                            vT_sb[b][mi].rearrange("p (h c) -> p h c", c=80)[0:msz, :, 0:48],
                            ps.rearrange("p (h c) -> p h c", c=48)[0:msz],
                            AF.Copy,
                        )

            # ---- phase B: attention ----
            with (
                tc.tile_pool(name="ebp", bufs=3) as ebp,
                tc.tile_pool(name="etp", bufs=2) as etp,
                tc.tile_pool(name="psqk", bufs=2, space="PSUM") as psqk,
                tc.tile_pool(name="psav", bufs=2, space="PSUM") as psav,
            ):
                eb_tiles = {}

                def attend(h, b):
                    eb_sb = eb_tiles[h]
                    av = psav.tile([65, 1024], f32, tag="av", name="avt")
                    for mi, (off, msz) in enumerate(MTS):
                        eT = etp.tile([128, N], f16, tag="eT", bufs=4, name="eTt")
                        qk = psqk.tile([128, 1024], f32, tag="qk", name="qkt")
                        for nch in range(2):
                            nc.tensor.matmul(
                                qk[0:msz, 512 * nch:512 * nch + NP],
                                k_sb[b][h][:, off:off + msz],
                                q_sb[b][h][:, NP * nch:NP * (nch + 1)],
                                start=True, stop=True,
                            )
                        nc.scalar.activation(
                            eT[0:msz].rearrange("p (c n) -> p c n", c=2),
                            qk[0:msz].rearrange("p (c n) -> p c n", n=512)[:, :, 0:NP],
                            AF.Exp)
                        # exp_bias multiply: split DVE / GPSIMD by m-tile
                        eng = nc.gpsimd if mi in (1, 4) else nc.vector
                        eng.tensor_tensor(
                            eT[0:msz, :], eT[0:msz, :], eb_sb[0:msz, mi, :], MUL)
                        for nch in range(2):
                            nc.tensor.matmul(
                                av[:, 512 * nch:512 * nch + NP],
                                vT_sb[b][mi][0:msz, 80 * h:80 * h + 65],
                                eT[0:msz, NP * nch:NP * (nch + 1)],
                                start=(mi == 0), stop=(mi == NMT - 1),
                            )
                    # evacuate out_unT + s row (f16)
                    nc.vector.tensor_copy(
                        ounT[b][:, h, :].rearrange("p (c n) -> p c n", c=2),
                        av.rearrange("p (c n) -> p c n", n=512)[:, :, 0:NP],
                    )

                def tail(b):
                    # batched softmax denominators + normalization
                    nc.gpsimd.dma_start(
                        sg_dram[b].rearrange("p n -> () (p n)").rearrange(
                            "() (h n) -> () h n", n=N),
                        ounT[b][64:65, :, :],
                    )
                    nc.gpsimd.dma_start(s_all[b][:], sg_dram[b][:])
                    nc.vector.reciprocal(r_all[b][:], s_all[b][:])
                    nc.vector.tensor_copy(r16[b][:], r_all[b][:])
                    nc.gpsimd.dma_start(r_dram[b][:], r16[b][:])
                    nc.gpsimd.dma_start(
                        bc_all[b][:],
                        r_dram[b].tensor.ap().rearrange(
                            "p n -> () (p n)").to_broadcast((48, NH * N)),
                    )
                    for h in range(NH):
                        prr, hpp = divmod(h, 2)
                        nc.vector.tensor_tensor(
                            onorm[b][prr][64 * hpp:64 * hpp + 48, :],
                            ounT[b][0:48, h, :],
                            bc_all[b][:, N * h:N * (h + 1)],
                            MUL,
                        )

                def proj(b):
                    for o in range(3):
                        ps = psav.tile([128, 1024], f32, tag="av", name="psy")
                        for nch in range(2):
                            for p in range(4):
                                nc.tensor.matmul(
                                    ps[:, 512 * nch:512 * nch + NP],
                                    pwT_sb[p][:, 128 * o:128 * (o + 1)],
                                    onorm[b][p][:, NP * nch:NP * (nch + 1)],
                                    start=(p == 0), stop=(p == 3),
                                )
                        y_sb = ypool.tile([128, N], f32, tag="y", name="ysb")
                        nc.scalar.activation(
                            y_sb.rearrange("p (c n) -> p c n", c=2),
                            ps.rearrange("p (c n) -> p c n", n=512)[:, :, 0:NP],
                            AF.Identity, bias=pb_sb[:, o:o + 1])
                        nc.sync.dma_start(
                            out_d[b, 128 * o:128 * (o + 1), :], y_sb[:])

                with tc.tile_pool(name="yp", bufs=3) as ypool:
                    # b=1 lags one head so b=0's tail overlaps b=1's last heads
                    sched = []
                    for h in range(NH):
                        sched.append((h, 0))
                        if h >= 1:
                            sched.append((h - 1, 1))
                    sched.append((NH - 1, 1))
                    for (h, b) in sched:
                        if b == 0 and h not in eb_tiles:
                            eb_sb = ebp.tile([128, NMT, N], f16, tag="eb", name="ebt")
                            for mi, (off, msz) in enumerate(MTS):
                                nc.sync.dma_start(eb_sb[0:msz, mi, :],
                                                  expb_d[h, off:off + msz, :])
                            eb_tiles[h] = eb_sb
                        attend(h, b)
                        if h == NH - 1:
                            tail(b)
                    proj(0)
                    proj(1)

    nc.finalize()
    return nc


def _prep_consts(q_w, q_b, kv_w, kv_b, proj_w, proj_b, attn_biases, bias_idxs):
    f16 = np.float16
    qw = (q_w * SCALE).astype(np.float32)
    qb = (q_b * SCALE).astype(np.float32)
    kw = kv_w[:C] * 0.5
    kb = kv_b[:C] * 0.5
    vw = kv_w[C:]
    vb = kv_b[C:]

    def pad64(w2, b1):  # [384(o), 384(c)] -> [512, 384] / [512]
        wp = np.zeros((512, C), np.float32)
        bp = np.zeros((512,), np.float32)
        for h in range(NH):
            wp[64 * h:64 * h + HD] = w2[HD * h:HD * (h + 1)]
            bp[64 * h:64 * h + HD] = b1[HD * h:HD * (h + 1)]
        return wp, bp

    qwp, qbp = pad64(qw, qb)
    kwp, kbp = pad64(kw, kb)
    qwT = np.ascontiguousarray(qwp.T.reshape(3, 128, 512)).astype(f16)
    kwT = np.ascontiguousarray(kwp.T.reshape(3, 128, 512)).astype(f16)
    vwT = np.ascontiguousarray(vw.T.reshape(3, 128, C)).astype(f16)

    # proj weights in onorm pair-tile layout: pair p row j -> channel
    pwT = np.zeros((4, 128, C), np.float32)
    for p in range(4):
        pwT[p, 0:HD] = proj_w[:, 96 * p:96 * p + HD].T
        pwT[p, 64:64 + HD] = proj_w[:, 96 * p + HD:96 * p + 96].T
    pwT = pwT.astype(f16)

    qb_h = np.ascontiguousarray(qbp.reshape(4, 128).T).astype(np.float32)
    kb_h = np.ascontiguousarray(kbp.reshape(4, 128).T).astype(np.float32)
    pb_h = np.ascontiguousarray(proj_b.reshape(3, 128).T).astype(np.float32)
    vb_h = vb.reshape(1, C).astype(f16)

    expb = np.ascontiguousarray(np.exp(attn_biases[:, bias_idxs]).astype(f16))

    vinit = np.zeros((128, 640), f16)
    vinit[:, 64::80] = 1.0

    return dict(qwT=qwT, kwT=kwT, vwT=vwT, pwT=pwT, qb=qb_h, kb=kb_h,
                vb=vb_h, pb=pb_h, expb=expb, vinit=vinit)


def kernel(ll, high_attn, q_w, q_b, kv_w, kv_b, proj_w, proj_b,
           attn_biases, bias_idxs):
    from concourse.bass_utils import run_bass_kernel_spmd

    global LAST_RESULTS
    ll = np.asarray(ll)
    high_attn = np.asarray(high_attn)

    if "nc" not in _CACHE:
        _CACHE["nc"] = _build_nc()
    nc = _CACHE["nc"]

    consts = _prep_consts(
        np.asarray(q_w), np.asarray(q_b), np.asarray(kv_w), np.asarray(kv_b),
        np.asarray(proj_w), np.asarray(proj_b), np.asarray(attn_biases),
        np.asarray(bias_idxs),
    )

    ll16 = ll.reshape(B, C, N).astype(np.float16)
    ha16 = high_attn.reshape(B, C, N).astype(np.float16)

    in_maps = []
    for i in range(NCORES):
        m = {"ll": ll16[BPC * i:BPC * (i + 1)], "ha": ha16[BPC * i:BPC * (i + 1)]}
        m.update(consts)
        in_maps.append(m)

    res = run_bass_kernel_spmd(nc, in_maps, core_ids=list(range(NCORES)),
                               trace=TRACE)
    LAST_RESULTS = {"exec_time_ns": res.exec_time_ns,
                    "scope_times": res.per_core_scope_times}

    out = np.empty((B, C, N), np.float32)
    for i in range(NCORES):
        out[BPC * i:BPC * (i + 1)] = res.results[i]["out"]
    return out.reshape(B, C, RES, RES)


# revision 14
# speedup vs baseline: 1.1886x; 1.0259x over previous
"""Trainium2 Bass kernel for nn_LowFreqCrossAttn (dense transformer cross-attention).

Data-parallel over batch: 16 batches -> 8 NeuronCores, 2 batches/core.
Weights / attention-bias tables replicated.

Per-core dataflow (all matmuls fp16 x fp16 -> f32 PSUM):
  A) q = (s*Wq) @ ll, k = (0.5*Wk) @ ha   (head rows duplicated to K=128:
     K<128 matmul streams never warm the PE HAM clock gate -> half clock)
     vT = ha^T @ WvT (+bias row)          (token-major, dense 48-col head
     blocks, re-laid to 80-col blocks with a ones col @64)
  B) per (head, batch): logitsT = k_h^T q_h  (m on partitions, n free)
     e = exp(logitsT) * exp_bias^T  (ACT exp -> f16, DVE/GPS mult; no
     max-subtraction: |logits + bias| <= 1.1 for this model)
     out_unT[d, n] (+ s row @ partition 64) = vT_slice^T @ e  (PSUM accum
     over m-tiles; QK + AV psum tiles span 2 banks so exp/evac run as one
     fat strided op per (h, b, m) instead of per chunk)
  B-tail, per batch) s rows -> DRAM bounce -> [16, 392] -> one batched DVE
     reciprocal -> DRAM bounce -> [1, 6272] -> gpsimd partition_broadcast
     out_norm = out_unT * (1/s)  (pair tiles, c-major, f16)
  C) y = WpT^T @ out_norm + b  (channel-major f32 out)
"""

import numpy as np

B = 16
C = 384
RES = 28
N = 784
NH = 8
HD = 48
NP = 392            # n-chunk (half of N; fits one PSUM bank in f32)
NCORES = 8
BPC = 2             # batches per core
SCALE = HD ** -0.5
# m-tiles: 6 x 128 + 1 x 16 (K=128 keeps the PE HAM warm)
MTS = [(128 * i, 128) for i in range(6)] + [(768, 16)]
NMT = len(MTS)

TRACE = False       # set True to capture an NTFF trace on core 0
LAST_RESULTS = {}   # exec_time_ns etc. from the last run (when TRACE)

_CACHE = {}


def _build_nc():
    import concourse.bacc as bacc
    import concourse.mybir as mybir
    import concourse.tile as tile

    f16 = mybir.dt.float16
    f32 = mybir.dt.float32
    AF = mybir.ActivationFunctionType
    MUL = mybir.AluOpType.mult

    nc = bacc.Bacc("TRN2", target_bir_lowering=False, debug=False)

    ll_d = nc.declare_dram_parameter("ll", [BPC, C, N], f16, isOutput=False)
    ha_d = nc.declare_dram_parameter("ha", [BPC, C, N], f16, isOutput=False)
    qwT_d = nc.declare_dram_parameter("qwT", [3, 128, 512], f16, isOutput=False)
    kwT_d = nc.declare_dram_parameter("kwT", [3, 128, 512], f16, isOutput=False)
    vwT_d = nc.declare_dram_parameter("vwT", [3, 128, 384], f16, isOutput=False)
    pwT_d = nc.declare_dram_parameter("pwT", [4, 128, 384], f16, isOutput=False)
    qb_d = nc.declare_dram_parameter("qb", [128, 4], f32, isOutput=False)
    kb_d = nc.declare_dram_parameter("kb", [128, 4], f32, isOutput=False)
    vb_d = nc.declare_dram_parameter("vb", [1, 384], f16, isOutput=False)
    pb_d = nc.declare_dram_parameter("pb", [128, 3], f32, isOutput=False)
    expb_d = nc.declare_dram_parameter("expb", [NH, N, N], f16, isOutput=False)
    vinit_d = nc.declare_dram_parameter("vinit", [128, 640], f16, isOutput=False)
    out_d = nc.declare_dram_parameter("out", [BPC, C, N], f32, isOutput=True)

    with tile.TileContext(nc) as tc:
        with (
            tc.tile_pool(name="const", bufs=1) as cp,
            tc.tile_pool(name="persist", bufs=1) as pp,
            tc.tile_pool(name="dram", bufs=1, space="DRAM") as dp,
        ):
            # ---- load constants ----
            qwT_sb = [cp.tile([128, 512], f16, tag=f"qwT{t}", name=f"qwT{t}") for t in range(3)]
            kwT_sb = [cp.tile([128, 512], f16, tag=f"kwT{t}", name=f"kwT{t}") for t in range(3)]
            vwT_sb = [cp.tile([128, 384], f16, tag=f"vwT{t}", name=f"vwT{t}") for t in range(3)]
            pwT_sb = [cp.tile([128, 384], f16, tag=f"pwT{p}", name=f"pwT{p}") for p in range(4)]
            for t in range(3):
                nc.sync.dma_start(qwT_sb[t][:], qwT_d[t])
            qb_sb = cp.tile([128, 4], f32, tag="qb", name="qb")
            kb_sb = cp.tile([128, 4], f32, tag="kb", name="kb")
            vb_sb = cp.tile([1, 384], f16, tag="vb", name="vb")
            pb_sb = cp.tile([128, 3], f32, tag="pb", name="pb")
            nc.sync.dma_start(qb_sb[:], qb_d[:])
            nc.sync.dma_start(kb_sb[:], kb_d[:])
            nc.sync.dma_start(vb_sb[:], vb_d[:])
            nc.sync.dma_start(pb_sb[:], pb_d[:])
            ones128 = cp.tile([1, 128], f16, tag="ones128", name="ones128")
            nc.gpsimd.memset(ones128[:], 1.0)

            # ---- persistent activation tiles ----
            q_sb = [[pp.tile([128, N], f16, tag=f"q{b}_{h}", name=f"q{b}_{h}")
                     for h in range(NH)] for b in range(BPC)]
            k_sb = [[pp.tile([128, N], f16, tag=f"k{b}_{h}", name=f"k{b}_{h}")
                     for h in range(NH)] for b in range(BPC)]
            vT_sb = [[pp.tile([128, 640], f16, tag=f"vT{b}_{m}", name=f"vT{b}_{m}")
                      for m in range(NMT)] for b in range(BPC)]
            ounT = [pp.tile([65, NH, N], f16, tag=f"ounT{b}", name=f"ounT{b}")
                    for b in range(BPC)]
            onorm = [[pp.tile([128, N], f16, tag=f"onorm{b}_{p}", name=f"onorm{b}_{p}")
                      for p in range(4)] for b in range(BPC)]
            s_all = [pp.tile([16, NP], f16, tag=f"s{b}", name=f"s{b}") for b in range(BPC)]
            r_all = [pp.tile([16, NP], f32, tag=f"r{b}", name=f"r{b}") for b in range(BPC)]
            r16 = [pp.tile([16, NP], f16, tag=f"r16{b}", name=f"r16{b}")
                   for b in range(BPC)]
            bc_all = [pp.tile([48, NH * N], f16, tag=f"bc{b}", name=f"bc{b}")
                      for b in range(BPC)]
            sg_dram = [dp.tile([16, NP], f16, tag=f"sg{b}", name=f"sg{b}")
                       for b in range(BPC)]
            r_dram = [dp.tile([16, NP], f16, tag=f"rd{b}", name=f"rd{b}")
                      for b in range(BPC)]

            # one-time layout init: vT 80-blocks (zeros + ones col @64) via DMA
            # const; onorm pad rows zeroed (32-aligned; data rows rewritten later)
            for b in range(BPC):
                for m in range(NMT):
                    nc.gpsimd.dma_start(vT_sb[b][m][:], vinit_d[:])
                for p in range(4):
                    nc.gpsimd.memset(onorm[b][p][32:64, :], 0.0)
                    nc.gpsimd.memset(onorm[b][p][96:128, :], 0.0)

            # ---- phase A: projections ----
            with (
                tc.tile_pool(name="actA", bufs=1) as apool,
                tc.tile_pool(name="psA", bufs=2, space="PSUM") as psA,
            ):
                ll_sb = [[apool.tile([128, N], f16, tag=f"ll{b}_{t}", name=f"ll{b}_{t}")
                          for t in range(3)] for b in range(BPC)]
                ha_sb = [[apool.tile([128, N], f16, tag=f"ha{b}_{t}", name=f"ha{b}_{t}")
                          for t in range(3)] for b in range(BPC)]
                for b in range(BPC):
                    for t in range(3):
                        nc.sync.dma_start(ll_sb[b][t][:], ll_d[b, 128 * t:128 * (t + 1), :])
                        nc.sync.dma_start(ha_sb[b][t][:], ha_d[b, 128 * t:128 * (t + 1), :])
                for t in range(3):
                    nc.sync.dma_start(kwT_sb[t][:], kwT_d[t])
                    nc.sync.dma_start(vwT_sb[t][:], vwT_d[t])
                for p in range(4):
                    nc.sync.dma_start(pwT_sb[p][:], pwT_d[p])
                for b in range(BPC):
                    # q / k projections -> head-pair tiles in q_sb[2p], then
                    # duplicate rows to build per-head K=128 tiles via DMAs
                    for (wt, bt, src_, dst) in (
                        (qwT_sb, qb_sb, ll_sb[b], q_sb[b]),
                        (kwT_sb, kb_sb, ha_sb[b], k_sb[b]),
                    ):
                        for p in range(4):
                            ps = psA.tile([128, 1024], f32, tag="qk", name="psqk")
                            for nch in range(2):
                                for t in range(3):
                                    nc.tensor.matmul(
                                        ps[:, 512 * nch:512 * nch + NP],
                                        wt[t][:, 128 * p:128 * (p + 1)],
                                        src_[t][:, NP * nch:NP * (nch + 1)],
                                        start=(t == 0),
                                        stop=(t == 2),
                                    )
                            nc.scalar.activation(
                                dst[2 * p].rearrange("p (c n) -> p c n", c=2),
                                ps.rearrange("p (c n) -> p c n", n=512)[:, :, 0:NP],
                                AF.Identity, bias=bt[:, p:p + 1],
                            )
                            nc.gpsimd.dma_start(dst[2 * p + 1][0:64, :], dst[2 * p][64:128, :])
                            nc.gpsimd.dma_start(dst[2 * p + 1][64:128, :], dst[2 * p][64:128, :])
                            nc.gpsimd.dma_start(dst[2 * p][64:128, :], dst[2 * p][0:64, :])
                    # vT projection -> dense 48-blocks, strided copy to 80-blocks
                    for mi, (off, msz) in enumerate(MTS):
                        ps = psA.tile([128, 384], f32, tag="vt", name="psvt")
                        nc.tensor.matmul(ps[0:msz, :], ones128[:, 0:msz], vb_sb[:],
                                         start=True, stop=False)
                        for t in range(3):
                            nc.tensor.matmul(
                                ps[0:msz, :],
                                ha_sb[b][t][:, off:off + msz],
                                vwT_sb[t][:],
                                start=False,
                                stop=(t == 2),
                            )
                        nc.scalar.activation(
                            vT_sb[b][mi].rearrange("p (h c) -> p h c", c=80)[0:msz, :, 0:48],
                            ps.rearrange("p (h c) -> p h c", c=48)[0:msz],
                            AF.Copy,
                        )

            # ---- phase B: attention ----
            with (
                tc.tile_pool(name="ebp", bufs=3) as ebp,
                tc.tile_pool(name="etp", bufs=2) as etp,
                tc.tile_pool(name="psqk", bufs=2, space="PSUM") as psqk,
                tc.tile_pool(name="psav", bufs=2, space="PSUM") as psav,
            ):
                eb_tiles = {}

                def attend(h, b):
                    eb_sb = eb_tiles[h]
                    av = psav.tile([65, 1024], f32, tag="av", name="avt")
                    for mi, (off, msz) in enumerate(MTS):
                        eT = etp.tile([128, N], f16, tag="eT", bufs=4, name="eTt")
                        qk = psqk.tile([128, 1024], f32, tag="qk", name="qkt")
                        for nch in range(2):
                            nc.tensor.matmul(
                                qk[0:msz, 512 * nch:512 * nch + NP],
                                k_sb[b][h][:, off:off + msz],
                                q_sb[b][h][:, NP * nch:NP * (nch + 1)],
                                start=True, stop=True,
                            )
                        nc.scalar.activation(
                            eT[0:msz].rearrange("p (c n) -> p c n", c=2),
                            qk[0:msz].rearrange("p (c n) -> p c n", n=512)[:, :, 0:NP],
                            AF.Exp)
                        nc.vector.tensor_tensor(
                            eT[0:msz, :], eT[0:msz, :], eb_sb[0:msz, mi, :], MUL)
                        for nch in range(2):
                            nc.tensor.matmul(
                                av[:, 512 * nch:512 * nch + NP],
                                vT_sb[b][mi][0:msz, 80 * h:80 * h + 65],
                                eT[0:msz, NP * nch:NP * (nch + 1)],
                                start=(mi == 0), stop=(mi == NMT - 1),
                            )
                    # evacuate out_unT + s row (f16)
                    nc.vector.tensor_copy(
                        ounT[b][:, h, :].rearrange("p (c n) -> p c n", c=2),
                        av.rearrange("p (c n) -> p c n", n=512)[:, :, 0:NP],
                    )

                def tail(b):
                    # batched softmax denominators + normalization
                    nc.gpsimd.dma_start(
                        sg_dram[b].rearrange("p n -> () (p n)").rearrange(
                            "() (h n) -> () h n", n=N),
                        ounT[b][64:65, :, :],
                    )
                    nc.gpsimd.dma_start(s_all[b][:], sg_dram[b][:])
                    nc.vector.reciprocal(r_all[b][:], s_all[b][:])
                    nc.vector.tensor_copy(r16[b][:], r_all[b][:])
                    nc.gpsimd.dma_start(r_dram[b][:], r16[b][:])
                    nc.gpsimd.dma_start(
                        bc_all[b][:],
                        r_dram[b].tensor.ap().rearrange(
                            "p n -> () (p n)").to_broadcast((48, NH * N)),
                    )
                    for h in range(NH):
                        prr, hpp = divmod(h, 2)
                        nc.vector.tensor_tensor(
                            onorm[b][prr][64 * hpp:64 * hpp + 48, :],
                            ounT[b][0:48, h, :],
                            bc_all[b][:, N * h:N * (h + 1)],
                            MUL,
                        )

                def proj(b):
                    for o in range(3):
                        ps = psav.tile([128, 1024], f32, tag="av", name="psy")
                        for nch in range(2):
                            for p in range(4):
                                nc.tensor.matmul(
                                    ps[:, 512 * nch:512 * nch + NP],
                                    pwT_sb[p][:, 128 * o:128 * (o + 1)],
                                    onorm[b][p][:, NP * nch:NP * (nch + 1)],
                                    start=(p == 0), stop=(p == 3),
                                )
                        y_sb = ypool.tile([128, N], f32, tag="y", name="ysb")
                        nc.scalar.activation(
                            y_sb.rearrange("p (c n) -> p c n", c=2),
                            ps.rearrange("p (c n) -> p c n", n=512)[:, :, 0:NP],
                            AF.Identity, bias=pb_sb[:, o:o + 1])
                        nc.sync.dma_start(
                            out_d[b, 128 * o:128 * (o + 1), :], y_sb[:])

                with tc.tile_pool(name="yp", bufs=3) as ypool:
                    # b=1 lags one head so b=0's tail overlaps b=1's last heads
                    sched = []
                    for h in range(NH):
                        sched.append((h, 0))
                        if h >= 1:
                            sched.append((h - 1, 1))
                    sched.append((NH - 1, 1))
                    for (h, b) in sched:
                        if b == 0 and h not in eb_tiles:
                            eb_sb = ebp.tile([128, NMT, N], f16, tag="eb", name="ebt")
                            for mi, (off, msz) in enumerate(MTS):
                                nc.sync.dma_start(eb_sb[0:msz, mi, :],
                                                  expb_d[h, off:off + msz, :])
                            eb_tiles[h] = eb_sb
                        attend(h, b)
                        if h == NH - 1:
                            tail(b)
                    proj(0)
                    proj(1)

    nc.finalize()
    return nc


def _prep_consts(q_w, q_b, kv_w, kv_b, proj_w, proj_b, attn_biases, bias_idxs):
    f16 = np.float16
    qw = (q_w * SCALE).astype(np.float32)
    qb = (q_b * SCALE).astype(np.float32)
    kw = kv_w[:C] * 0.5
    kb = kv_b[:C] * 0.5
    vw = kv_w[C:]
    vb = kv_b[C:]

    def pad64(w2, b1):  # [384(o), 384(c)] -> [512, 384] / [512]
        wp = np.zeros((512, C), np.float32)
        bp = np.zeros((512,), np.float32)
        for h in range(NH):
            wp[64 * h:64 * h + HD] = w2[HD * h:HD * (h + 1)]
            bp[64 * h:64 * h + HD] = b1[HD * h:HD * (h + 1)]
        return wp, bp

    qwp, qbp = pad64(qw, qb)
    kwp, kbp = pad64(kw, kb)
    qwT = np.ascontiguousarray(qwp.T.reshape(3, 128, 512)).astype(f16)
    kwT = np.ascontiguousarray(kwp.T.reshape(3, 128, 512)).astype(f16)
    vwT = np.ascontiguousarray(vw.T.reshape(3, 128, C)).astype(f16)

    # proj weights in onorm pair-tile layout: pair p row j -> channel
    pwT = np.zeros((4, 128, C), np.float32)
    for p in range(4):
        pwT[p, 0:HD] = proj_w[:, 96 * p:96 * p + HD].T
        pwT[p, 64:64 + HD] = proj_w[:, 96 * p + HD:96 * p + 96].T
    pwT = pwT.astype(f16)

    qb_h = np.ascontiguousarray(qbp.reshape(4, 128).T).astype(np.float32)
    kb_h = np.ascontiguousarray(kbp.reshape(4, 128).T).astype(np.float32)
    pb_h = np.ascontiguousarray(proj_b.reshape(3, 128).T).astype(np.float32)
    vb_h = vb.reshape(1, C).astype(f16)

    expb = np.ascontiguousarray(np.exp(attn_biases[:, bias_idxs]).astype(f16))

    vinit = np.zeros((128, 640), f16)
    vinit[:, 64::80] = 1.0

    return dict(qwT=qwT, kwT=kwT, vwT=vwT, pwT=pwT, qb=qb_h, kb=kb_h,
                vb=vb_h, pb=pb_h, expb=expb, vinit=vinit)


def kernel(ll, high_attn, q_w, q_b, kv_w, kv_b, proj_w, proj_b,
           attn_biases, bias_idxs):
    from concourse.bass_utils import run_bass_kernel_spmd

    global LAST_RESULTS
    ll = np.asarray(ll)
    high_attn = np.asarray(high_attn)

    if "nc" not in _CACHE:
        _CACHE["nc"] = _build_nc()
    nc = _CACHE["nc"]

    consts = _prep_consts(
        np.asarray(q_w), np.asarray(q_b), np.asarray(kv_w), np.asarray(kv_b),
        np.asarray(proj_w), np.asarray(proj_b), np.asarray(attn_biases),
        np.asarray(bias_idxs),
    )

    ll16 = ll.reshape(B, C, N).astype(np.float16)
    ha16 = high_attn.reshape(B, C, N).astype(np.float16)

    in_maps = []
    for i in range(NCORES):
        m = {"ll": ll16[BPC * i:BPC * (i + 1)], "ha": ha16[BPC * i:BPC * (i + 1)]}
        m.update(consts)
        in_maps.append(m)

    res = run_bass_kernel_spmd(nc, in_maps, core_ids=list(range(NCORES)),
                               trace=TRACE)
    LAST_RESULTS = {"exec_time_ns": res.exec_time_ns,
                    "scope_times": res.per_core_scope_times}

    out = np.empty((B, C, N), np.float32)
    for i in range(NCORES):
        out[BPC * i:BPC * (i + 1)] = res.results[i]["out"]
    return out.reshape(B, C, RES, RES)


# revision 16
# speedup vs baseline: 1.3487x; 1.1347x over previous
"""Trainium2 Bass kernel for nn_LowFreqCrossAttn (dense transformer cross-attention).

Data-parallel over batch: 16 batches -> 8 NeuronCores, 2 batches/core.
Weights / attention-bias tables replicated.

Per-core dataflow (all matmuls fp16 x fp16 -> f32 PSUM):
  A) q = (s*Wq) @ ll, k = (0.5*Wk) @ ha   (head rows duplicated to K=128:
     K<128 matmul streams never warm the PE HAM clock gate -> half clock)
     vT = ha^T @ WvT (+bias row)          (token-major, dense 48-col head
     blocks, re-laid to 80-col blocks with a ones col @64)
  B) per (head, batch): logitsT = k_h^T q_h  (m on partitions, n free)
     e = exp(logitsT) * exp_bias^T  (ACT exp -> f16, DVE/GPS mult; no
     max-subtraction: |logits + bias| <= 1.1 for this model)
     out_unT[d, n] (+ s row @ partition 64) = vT_slice^T @ e  (PSUM accum
     over m-tiles; QK + AV psum tiles span 2 banks so exp/evac run as one
     fat strided op per (h, b, m) instead of per chunk)
  B-tail, per batch) s rows -> DRAM bounce -> [16, 392] -> one batched DVE
     reciprocal -> DRAM bounce -> [1, 6272] -> gpsimd partition_broadcast
     out_norm = out_unT * (1/s)  (pair tiles, c-major, f16)
  C) y = WpT^T @ out_norm + b  (channel-major f32 out)
"""

import numpy as np

B = 16
C = 384
RES = 28
N = 784
NH = 8
HD = 48
NP = 392            # n-chunk (half of N; fits one PSUM bank in f32)
NCORES = 8
BPC = 2             # batches per core
SCALE = HD ** -0.5
# m-tiles: 6 x 128 + 1 x 16 (K=128 keeps the PE HAM warm)
MTS = [(128 * i, 128) for i in range(6)] + [(768, 16)]
NMT = len(MTS)

TRACE = False       # set True to capture an NTFF trace on core 0
LAST_RESULTS = {}   # exec_time_ns etc. from the last run (when TRACE)

_CACHE = {}


def _build_nc():
    import concourse.bacc as bacc
    import concourse.mybir as mybir
    import concourse.tile as tile

    f16 = mybir.dt.float16
    f32 = mybir.dt.float32
    AF = mybir.ActivationFunctionType
    MUL = mybir.AluOpType.mult

    nc = bacc.Bacc("TRN2", target_bir_lowering=False, debug=False)

    ll_d = nc.declare_dram_parameter("ll", [BPC, C, N], f16, isOutput=False)
    ha_d = nc.declare_dram_parameter("ha", [BPC, C, N], f16, isOutput=False)
    qwT_d = nc.declare_dram_parameter("qwT", [3, 128, 512], f16, isOutput=False)
    kwT_d = nc.declare_dram_parameter("kwT", [3, 128, 512], f16, isOutput=False)
    vwT_d = nc.declare_dram_parameter("vwT", [3, 128, 384], f16, isOutput=False)
    pwT_d = nc.declare_dram_parameter("pwT", [4, 128, 384], f16, isOutput=False)
    qb_d = nc.declare_dram_parameter("qb", [128, 4], f32, isOutput=False)
    kb_d = nc.declare_dram_parameter("kb", [128, 4], f32, isOutput=False)
    vb_d = nc.declare_dram_parameter("vb", [1, 384], f16, isOutput=False)
    pb_d = nc.declare_dram_parameter("pb", [128, 3], f32, isOutput=False)
    expb_d = nc.declare_dram_parameter("expb", [NH, N, N], f16, isOutput=False)
    vinit_d = nc.declare_dram_parameter("vinit", [128, 640], f16, isOutput=False)
    out_d = nc.declare_dram_parameter("out", [BPC, C, N], f32, isOutput=True)

    with tile.TileContext(nc) as tc:
        with (
            tc.tile_pool(name="const", bufs=1) as cp,
            tc.tile_pool(name="persist", bufs=1) as pp,
            tc.tile_pool(name="dram", bufs=1, space="DRAM") as dp,
        ):
            # ---- load constants ----
            qwT_sb = [cp.tile([128, 512], f16, tag=f"qwT{t}", name=f"qwT{t}") for t in range(3)]
            kwT_sb = [cp.tile([128, 512], f16, tag=f"kwT{t}", name=f"kwT{t}") for t in range(3)]
            vwT_sb = [cp.tile([128, 384], f16, tag=f"vwT{t}", name=f"vwT{t}") for t in range(3)]
            pwT_sb = [cp.tile([128, 384], f16, tag=f"pwT{p}", name=f"pwT{p}") for p in range(4)]
            for t in range(3):
                nc.sync.dma_start(qwT_sb[t][:], qwT_d[t])
            qb_sb = cp.tile([128, 4], f32, tag="qb", name="qb")
            kb_sb = cp.tile([128, 4], f32, tag="kb", name="kb")
            vb_sb = cp.tile([1, 384], f16, tag="vb", name="vb")
            pb_sb = cp.tile([128, 3], f32, tag="pb", name="pb")
            nc.sync.dma_start(qb_sb[:], qb_d[:])
            nc.sync.dma_start(kb_sb[:], kb_d[:])
            nc.sync.dma_start(vb_sb[:], vb_d[:])
            nc.sync.dma_start(pb_sb[:], pb_d[:])
            ones128 = cp.tile([1, 128], f16, tag="ones128", name="ones128")
            nc.gpsimd.memset(ones128[:], 1.0)

            # ---- persistent activation tiles ----
            q_sb = [[pp.tile([128, N], f16, tag=f"q{b}_{h}", name=f"q{b}_{h}")
                     for h in range(NH)] for b in range(BPC)]
            k_sb = [[pp.tile([128, N], f16, tag=f"k{b}_{h}", name=f"k{b}_{h}")
                     for h in range(NH)] for b in range(BPC)]
            vT_sb = [[pp.tile([128, 640], f16, tag=f"vT{b}_{m}", name=f"vT{b}_{m}")
                      for m in range(NMT)] for b in range(BPC)]
            ounT = [pp.tile([65, NH, N], f16, tag=f"ounT{b}", name=f"ounT{b}")
                    for b in range(BPC)]
            onorm = [[pp.tile([128, N], f16, tag=f"onorm{b}_{p}", name=f"onorm{b}_{p}")
                      for p in range(4)] for b in range(BPC)]
            s_all = [pp.tile([12, NP], f16, tag=f"s{b}", name=f"s{b}") for b in range(BPC)]
            s32 = [pp.tile([12, NP], f32, tag=f"s32{b}", name=f"s32{b}") for b in range(BPC)]
            r_all = [pp.tile([12, NP], f32, tag=f"r{b}", name=f"r{b}") for b in range(BPC)]
            r16 = [pp.tile([12, NP], f16, tag=f"r16{b}", name=f"r16{b}")
                   for b in range(BPC)]
            s2_all = [pp.tile([4, NP], f16, tag=f"s2{b}", name=f"s2{b}") for b in range(BPC)]
            s2_32 = [pp.tile([4, NP], f32, tag=f"s232{b}", name=f"s232{b}") for b in range(BPC)]
            r2_all = [pp.tile([4, NP], f32, tag=f"r2{b}", name=f"r2{b}") for b in range(BPC)]
            r2_16 = [pp.tile([4, NP], f16, tag=f"r216{b}", name=f"r216{b}")
                     for b in range(BPC)]
            bc_all = [pp.tile([48, NH * N], f16, tag=f"bc{b}", name=f"bc{b}")
                      for b in range(BPC)]
            sg_dram = [dp.tile([12, NP], f16, tag=f"sg{b}", name=f"sg{b}")
                       for b in range(BPC)]
            r_dram = [dp.tile([12, NP], f16, tag=f"rd{b}", name=f"rd{b}")
                      for b in range(BPC)]
            sg2_dram = [dp.tile([4, NP], f16, tag=f"sg2{b}", name=f"sg2{b}")
                        for b in range(BPC)]
            r2_dram = [dp.tile([4, NP], f16, tag=f"rd2{b}", name=f"rd2{b}")
                       for b in range(BPC)]

            # one-time layout init: vT 80-blocks (zeros + ones col @64) via DMA
            # const; onorm pad rows zeroed (32-aligned; data rows rewritten later)
            for b in range(BPC):
                for m in range(NMT):
                    nc.gpsimd.dma_start(vT_sb[b][m][:], vinit_d[:])
                for p in range(4):
                    nc.gpsimd.memset(onorm[b][p][32:64, :], 0.0)
                    nc.gpsimd.memset(onorm[b][p][96:128, :], 0.0)

            # ---- phase A: projections ----
            with (
                tc.tile_pool(name="actA", bufs=1) as apool,
                tc.tile_pool(name="psA", bufs=2, space="PSUM") as psA,
            ):
                ll_sb = [[apool.tile([128, N], f16, tag=f"ll{b}_{t}", name=f"ll{b}_{t}")
                          for t in range(3)] for b in range(BPC)]
                ha_sb = [[apool.tile([128, N], f16, tag=f"ha{b}_{t}", name=f"ha{b}_{t}")
                          for t in range(3)] for b in range(BPC)]
                for b in range(BPC):
                    for t in range(3):
                        nc.sync.dma_start(ll_sb[b][t][:], ll_d[b, 128 * t:128 * (t + 1), :])
                        nc.sync.dma_start(ha_sb[b][t][:], ha_d[b, 128 * t:128 * (t + 1), :])
                for t in range(3):
                    nc.sync.dma_start(kwT_sb[t][:], kwT_d[t])
                    nc.sync.dma_start(vwT_sb[t][:], vwT_d[t])
                for p in range(4):
                    nc.sync.dma_start(pwT_sb[p][:], pwT_d[p])
                for b in range(BPC):
                    # q / k projections -> head-pair tiles in q_sb[2p], then
                    # duplicate rows to build per-head K=128 tiles via DMAs
                    for (wt, bt, src_, dst) in (
                        (qwT_sb, qb_sb, ll_sb[b], q_sb[b]),
                        (kwT_sb, kb_sb, ha_sb[b], k_sb[b]),
                    ):
                        for p in range(4):
                            ps = psA.tile([128, 1024], f32, tag="qk", name="psqk")
                            for nch in range(2):
                                for t in range(3):
                                    nc.tensor.matmul(
                                        ps[:, 512 * nch:512 * nch + NP],
                                        wt[t][:, 128 * p:128 * (p + 1)],
                                        src_[t][:, NP * nch:NP * (nch + 1)],
                                        start=(t == 0),
                                        stop=(t == 2),
                                    )
                            nc.scalar.activation(
                                dst[2 * p].rearrange("p (c n) -> p c n", c=2),
                                ps.rearrange("p (c n) -> p c n", n=512)[:, :, 0:NP],
                                AF.Identity, bias=bt[:, p:p + 1],
                            )
                            nc.gpsimd.dma_start(dst[2 * p + 1][0:64, :], dst[2 * p][64:128, :])
                            nc.gpsimd.dma_start(dst[2 * p + 1][64:128, :], dst[2 * p][64:128, :])
                            nc.gpsimd.dma_start(dst[2 * p][64:128, :], dst[2 * p][0:64, :])
                    # vT projection -> dense 48-blocks, strided copy to 80-blocks
                    for mi, (off, msz) in enumerate(MTS):
                        ps = psA.tile([128, 384], f32, tag="vt", name="psvt")
                        nc.tensor.matmul(ps[0:msz, :], ones128[:, 0:msz], vb_sb[:],
                                         start=True, stop=False)
                        for t in range(3):
                            nc.tensor.matmul(
                                ps[0:msz, :],
                                ha_sb[b][t][:, off:off + msz],
                                vwT_sb[t][:],
                                start=False,
                                stop=(t == 2),
                            )
                        nc.scalar.activation(
                            vT_sb[b][mi].rearrange("p (h c) -> p h c", c=80)[0:msz, :, 0:48],
                            ps.rearrange("p (h c) -> p h c", c=48)[0:msz],
                            AF.Copy,
                        )

            # ---- phase B: attention ----
            with (
                tc.tile_pool(name="ebp", bufs=3) as ebp,
                tc.tile_pool(name="etp", bufs=2) as etp,
                tc.tile_pool(name="psqk", bufs=2, space="PSUM") as psqk,
                tc.tile_pool(name="psav", bufs=2, space="PSUM") as psav,
            ):
                eb_tiles = {}

                def attend(h, b):
                    eb_sb = eb_tiles[h]
                    av = psav.tile([65, 1024], f32, tag="av", name="avt")
                    for mi, (off, msz) in enumerate(MTS):
                        eT = etp.tile([128, N], f16, tag="eT", bufs=4, name="eTt")
                        qk = psqk.tile([128, 1024], f32, tag="qk", name="qkt")
                        for nch in range(2):
                            nc.tensor.matmul(
                                qk[0:msz, 512 * nch:512 * nch + NP],
                                k_sb[b][h][:, off:off + msz],
                                q_sb[b][h][:, NP * nch:NP * (nch + 1)],
                                start=True, stop=True,
                            )
                        nc.scalar.activation(
                            eT[0:msz].rearrange("p (c n) -> p c n", c=2),
                            qk[0:msz].rearrange("p (c n) -> p c n", n=512)[:, :, 0:NP],
                            AF.Exp)
                        nc.vector.tensor_tensor(
                            eT[0:msz, :], eT[0:msz, :], eb_sb[0:msz, mi, :], MUL)
                        for nch in range(2):
                            nc.tensor.matmul(
                                av[:, 512 * nch:512 * nch + NP],
                                vT_sb[b][mi][0:msz, 80 * h:80 * h + 65],
                                eT[0:msz, NP * nch:NP * (nch + 1)],
                                start=(mi == 0), stop=(mi == NMT - 1),
                            )
                    # evacuate out_unT + s row (f16)
                    nc.vector.tensor_copy(
                        ounT[b][:, h, :].rearrange("p (c n) -> p c n", c=2),
                        av.rearrange("p (c n) -> p c n", n=512)[:, :, 0:NP],
                    )

                def tail1(b):
                    # heads 0-5: softmax denominators ready after (5, b)
                    nc.gpsimd.dma_start(
                        sg_dram[b].rearrange("p n -> () (p n)").rearrange(
                            "() (h n) -> () h n", n=N),
                        ounT[b][64:65, 0:6, :],
                    )
                    nc.gpsimd.dma_start(s_all[b][:], sg_dram[b][:])
                    nc.vector.tensor_copy(s32[b][:], s_all[b][:])
                    nc.vector.reciprocal_approx_fast(r_all[b][:], s32[b][:])
                    nc.vector.tensor_copy(r16[b][:], r_all[b][:])
                    nc.gpsimd.dma_start(r_dram[b][:], r16[b][:])
                    nc.gpsimd.dma_start(
                        bc_all[b][:, 0:6 * N],
                        r_dram[b].tensor.ap().rearrange(
                            "p n -> () (p n)").to_broadcast((48, 6 * N)),
                    )
                    for h in range(6):
                        prr, hpp = divmod(h, 2)
                        nc.vector.tensor_tensor(
                            onorm[b][prr][64 * hpp:64 * hpp + 48, :],
                            ounT[b][0:48, h, :],
                            bc_all[b][:, N * h:N * (h + 1)],
                            MUL,
                        )

                def tail2(b):
                    # heads 6-7 after (7, b)
                    nc.gpsimd.dma_start(
                        sg2_dram[b].rearrange("p n -> () (p n)").rearrange(
                            "() (h n) -> () h n", n=N),
                        ounT[b][64:65, 6:8, :],
                    )
                    nc.gpsimd.dma_start(s2_all[b][:], sg2_dram[b][:])
                    nc.vector.tensor_copy(s2_32[b][:], s2_all[b][:])
                    nc.vector.reciprocal_approx_fast(r2_all[b][:], s2_32[b][:])
                    nc.vector.tensor_copy(r2_16[b][:], r2_all[b][:])
                    nc.gpsimd.dma_start(r2_dram[b][:], r2_16[b][:])
                    nc.gpsimd.dma_start(
                        bc_all[b][:, 6 * N:8 * N],
                        r2_dram[b].tensor.ap().rearrange(
                            "p n -> () (p n)").to_broadcast((48, 2 * N)),
                    )
                    for h in (6, 7):
                        prr, hpp = divmod(h, 2)
                        nc.vector.tensor_tensor(
                            onorm[b][prr][64 * hpp:64 * hpp + 48, :],
                            ounT[b][0:48, h, :],
                            bc_all[b][:, N * h:N * (h + 1)],
                            MUL,
                        )

                def proj(b):
                    for o in range(3):
                        ps = psav.tile([128, 1024], f32, tag="av", name="psy")
                        for nch in range(2):
                            for p in range(4):
                                nc.tensor.matmul(
                                    ps[:, 512 * nch:512 * nch + NP],
                                    pwT_sb[p][:, 128 * o:128 * (o + 1)],
                                    onorm[b][p][:, NP * nch:NP * (nch + 1)],
                                    start=(p == 0), stop=(p == 3),
                                )
                        y_sb = ypool.tile([128, N], f32, tag="y", name="ysb")
                        nc.scalar.activation(
                            y_sb.rearrange("p (c n) -> p c n", c=2),
                            ps.rearrange("p (c n) -> p c n", n=512)[:, :, 0:NP],
                            AF.Identity, bias=pb_sb[:, o:o + 1])
                        nc.sync.dma_start(
                            out_d[b, 128 * o:128 * (o + 1), :], y_sb[:])

                with tc.tile_pool(name="yp", bufs=3) as ypool:
                    # b=1 lags one head so b=0's tail overlaps b=1's last heads
                    sched = []
                    for h in range(NH):
                        sched.append((h, 0))
                        if h >= 1:
                            sched.append((h - 1, 1))
                    sched.append((NH - 1, 1))
                    for (h, b) in sched:
                        if b == 0 and h not in eb_tiles:
                            eb_sb = ebp.tile([128, NMT, N], f16, tag="eb", name="ebt")
                            for mi, (off, msz) in enumerate(MTS):
                                nc.sync.dma_start(eb_sb[0:msz, mi, :],
                                                  expb_d[h, off:off + msz, :])
                            eb_tiles[h] = eb_sb
                        attend(h, b)
                        if h == 5:
                            tail1(b)
                        if h == NH - 1:
                            tail2(b)
                            proj(b)

    nc.finalize()
    return nc


def _prep_consts(q_w, q_b, kv_w, kv_b, proj_w, proj_b, attn_biases, bias_idxs):
    f16 = np.float16
    qw = (q_w * SCALE).astype(np.float32)
    qb = (q_b * SCALE).astype(np.float32)
    kw = kv_w[:C] * 0.5
    kb = kv_b[:C] * 0.5
    vw = kv_w[C:]
    vb = kv_b[C:]

    def pad64(w2, b1):  # [384(o), 384(c)] -> [512, 384] / [512]
        wp = np.zeros((512, C), np.float32)
        bp = np.zeros((512,), np.float32)
        for h in range(NH):
            wp[64 * h:64 * h + HD] = w2[HD * h:HD * (h + 1)]
            bp[64 * h:64 * h + HD] = b1[HD * h:HD * (h + 1)]
        return wp, bp

    qwp, qbp = pad64(qw, qb)
    kwp, kbp = pad64(kw, kb)
    qwT = np.ascontiguousarray(qwp.T.reshape(3, 128, 512)).astype(f16)
    kwT = np.ascontiguousarray(kwp.T.reshape(3, 128, 512)).astype(f16)
    vwT = np.ascontiguousarray(vw.T.reshape(3, 128, C)).astype(f16)

    # proj weights in onorm pair-tile layout: pair p row j -> channel
    pwT = np.zeros((4, 128, C), np.float32)
    for p in range(4):
        pwT[p, 0:HD] = proj_w[:, 96 * p:96 * p + HD].T
        pwT[p, 64:64 + HD] = proj_w[:, 96 * p + HD:96 * p + 96].T
    pwT = pwT.astype(f16)

    qb_h = np.ascontiguousarray(qbp.reshape(4, 128).T).astype(np.float32)
    kb_h = np.ascontiguousarray(kbp.reshape(4, 128).T).astype(np.float32)
    pb_h = np.ascontiguousarray(proj_b.reshape(3, 128).T).astype(np.float32)
    vb_h = vb.reshape(1, C).astype(f16)

    expb = np.ascontiguousarray(np.exp(attn_biases[:, bias_idxs]).astype(f16))

    vinit = np.zeros((128, 640), f16)
    vinit[:, 64::80] = 1.0

    return dict(qwT=qwT, kwT=kwT, vwT=vwT, pwT=pwT, qb=qb_h, kb=kb_h,
                vb=vb_h, pb=pb_h, expb=expb, vinit=vinit)


def kernel(ll, high_attn, q_w, q_b, kv_w, kv_b, proj_w, proj_b,
           attn_biases, bias_idxs):
    from concourse.bass_utils import run_bass_kernel_spmd

    global LAST_RESULTS
    ll = np.asarray(ll)
    high_attn = np.asarray(high_attn)

    if "nc" not in _CACHE:
        _CACHE["nc"] = _build_nc()
    nc = _CACHE["nc"]

    consts = _prep_consts(
        np.asarray(q_w), np.asarray(q_b), np.asarray(kv_w), np.asarray(kv_b),
        np.asarray(proj_w), np.asarray(proj_b), np.asarray(attn_biases),
        np.asarray(bias_idxs),
    )

    ll16 = ll.reshape(B, C, N).astype(np.float16)
    ha16 = high_attn.reshape(B, C, N).astype(np.float16)

    in_maps = []
    for i in range(NCORES):
        m = {"ll": ll16[BPC * i:BPC * (i + 1)], "ha": ha16[BPC * i:BPC * (i + 1)]}
        m.update(consts)
        in_maps.append(m)

    res = run_bass_kernel_spmd(nc, in_maps, core_ids=list(range(NCORES)),
                               trace=TRACE)
    LAST_RESULTS = {"exec_time_ns": res.exec_time_ns,
                    "scope_times": res.per_core_scope_times}

    out = np.empty((B, C, N), np.float32)
    for i in range(NCORES):
        out[BPC * i:BPC * (i + 1)] = res.results[i]["out"]
    return out.reshape(B, C, RES, RES)
